# revision 30
# baseline (speedup 1.0000x reference)
"""Distributed GQA attention kernel for Trainium2 (8 NeuronCores).

Sharding: 2-way data parallel over batch x 4-way tensor parallel over heads.
Core c handles batch b = c // 4 and head group g = c % 4 (8 q-heads, 2 kv-heads).
Each core computes a full-size partial of the output (its head group pushed
through Wo); the host sums the 4 partials per batch. No on-device collective.

Device-side layout is feature-major (Q^T/K^T: [feature partitions, T free]) so
projections consume the host-pre-transposed x^T directly, attention scores are
computed transposed (S^T[tk, tq]) so softmax(P)@V needs no transposes, and the
softmax denominator is broadcast for free by 64 ones-columns appended to V
(the PV matmul then emits 64 identical sum-exp rows; normalization is a copy +
one [64,1024] reciprocal + fused multiplies out of PSUM; reciprocal must read
SBUF - the custom-DVE op returns garbage on a partition-offset PSUM source).
Score matmuls are K=64 so the two heads' matmuls land on different PE row
groups (base partitions 0/64) and stream CONCURRENTLY (2x packing).

Schedule: the kernel is PE-streaming-bound (~224us of matmul columns at
2.4GHz) with the scalar engine (exp: (N+352)/1.2ns per instruction, ~163us
total) second. The emission is a dependency wavefront: x arrives on the two
HWDGE rings (sync+scalar — the only engines with hardware DGE); ~80 dummy
matmuls warm the PE HAM clock-gate (cold=1.2GHz) while the first slices land;
V/K/Q projections chase the slices; attention for query-tile qt starts as
soon as its K/Q columns exist. Attention inner loops pull cost-metered
"filler" PE work (remaining projections, Wo output blocks) from a queue
between score/PV pairs so the PE never idles while ACT chews exp: the ACT
deficit is ~1us per 2-key-block iteration. ACT runs nothing but exp — rope's
psum cast runs on DVE and rotate-half is a PE matmul against a constant
128x128 permutation (software-pipelined behind the cast via flush_rope, so
the PE never waits on DVE). qt region order 0,1,3,2 balances each region's
exp load against available filler (wave3 projections must drain during qt1
since qt3 needs them; wave2 defers into the qt3 region as filler); Wo(qt2)
plus its output DMA is the only tail. Output y rides the idle sync engine
only — a dma_start occupies its issuing engine ~5ns/descriptor, which would
starve exp if placed on scalar.

Note: the chip randomly enters a ~2.0GHz power state (vs 2.4) for whole runs;
measured times swing ~15% run-to-run on identical code.
"""

import numpy as np
import ml_dtypes
from collections import deque
from contextlib import ExitStack

import concourse.bass as bass
from concourse import bacc
import concourse.mybir as mybir
import concourse.tile as tile
from concourse.bass_utils import run_bass_kernel_spmd

BF16 = mybir.dt.bfloat16
F32 = mybir.dt.float32
AF = mybir.ActivationFunctionType

P = 128
B, T, D = 2, 2048, 2048
NUM_HEADS, NUM_KV_HEADS, HD = 32, 8, 64
FQ = 512          # q features per core (8 heads x 64)
DKV = 128         # kv features per core (2 kv heads x 64)
KO = D // P       # 16 contraction tiles over d_model
NT = T // 512     # 4 tiles of 512 along T
NXE = 8           # x arrives in 8 T-slices of 256
TE = T // NXE
SCALE = 1.0 / np.sqrt(HD)
ROPE_BASE = 10000.0
# local head order inside the 512 q-features: pairs (j, j+4) so that the two
# heads in partition tile j sit at bases 0/64 matching kv heads 0/1 in K^T
PERM_Q = [0, 4, 1, 5, 2, 6, 3, 7]

_nc_cache = {}


def build_nc():
    if "nc" in _nc_cache:
        return _nc_cache["nc"]
    nc = bacc.Bacc()
    # host-packed layouts: row = slice*128 + partition, all loads contiguous
    xS = nc.declare_dram_parameter("xS", [NXE * P, KO * TE], BF16, isOutput=False)
    wqS = nc.declare_dram_parameter("wqS", [4 * P, KO * P], BF16, isOutput=False)
    wkS = nc.declare_dram_parameter("wkS", [P, KO * DKV], BF16, isOutput=False)
    wvS = nc.declare_dram_parameter("wvS", [P, KO * DKV], BF16, isOutput=False)
    woS = nc.declare_dram_parameter("woS", [P, 4 * D], BF16, isOutput=False)
    cosd = nc.declare_dram_parameter("cosT", [P, T], BF16, isOutput=False)
    sind = nc.declare_dram_parameter("sinT", [P, T], BF16, isOutput=False)
    mskd = nc.declare_dram_parameter("tri", [P, P], BF16, isOutput=False)
    prmd = nc.declare_dram_parameter("prm", [P, P], BF16, isOutput=False)
    y = nc.declare_dram_parameter("y", [T, D], BF16, isOutput=True)

    with tile.TileContext(nc) as tc:
        with ExitStack() as ctx:
            const = ctx.enter_context(tc.tile_pool(name="const", bufs=1))
            work = ctx.enter_context(tc.tile_pool(name="work", bufs=6))
            otp = ctx.enter_context(tc.tile_pool(name="otp", bufs=2))
            pexp = ctx.enter_context(tc.tile_pool(name="pexp", bufs=8))
            rrp = ctx.enter_context(tc.tile_pool(name="rrp", bufs=2))
            ysp = ctx.enter_context(tc.tile_pool(name="ysp", bufs=2))
            big_ps = ctx.enter_context(tc.tile_pool(name="bigps", bufs=2, space="PSUM"))
            pv_ps = ctx.enter_context(tc.tile_pool(name="pvps", bufs=1, space="PSUM"))
            s_ps = ctx.enter_context(tc.tile_pool(name="sps", bufs=2, space="PSUM"))

            x_sb = const.tile([P, NXE, KO, TE], BF16, tag="x")
            wq_sb = const.tile([P, 4, KO, P], BF16, tag="wq")
            wv_sb = const.tile([P, KO, DKV], BF16, tag="wv")
            wk_sb = const.tile([P, KO, DKV], BF16, tag="wk")
            cos_sb = const.tile([P, T], BF16, tag="cos")
            sin_sb = const.tile([P, T], BF16, tag="sin")
            tri_sb = const.tile([P, P], BF16, tag="tri")
            prm_sb = const.tile([P, P], BF16, tag="prm")
            wo_sb = const.tile([P, 4, D], BF16, tag="wo")
            warm_sb = const.tile([P, P], BF16, tag="warm")

            # ---- V layout + PE warm-up (emitted first: gpsimd memsets, then
            # dummy matmuls keep the PE busy from ~6.4us so the HAM clock-gate
            # reaches 8/8 before real data lands, and stays there) ----
            v_sb = const.tile([P, 16, 256], BF16, tag="v")
            nc.gpsimd.memset(warm_sb[:], 0.0)
            nc.gpsimd.memset(v_sb[:, :, 64:128], 1.0)
            nc.gpsimd.memset(v_sb[:, :, 192:256], 1.0)
            wps = big_ps.tile([P, P], F32, tag="big")
            for _ in range(82):
                nc.tensor.matmul(wps[:], warm_sb[:], warm_sb[:],
                                 start=True, stop=True)

            # ---- input loads: two HWDGE rings (sync + gpsimd), x slices
            # split into partition halves so both rings carry every slice;
            # weights slotted just before first need ----
            def xs_half(e, h):
                return xS[e * P + 64 * h:e * P + 64 * (h + 1), :].rearrange(
                    "p (k t) -> p k t", k=KO)

            def wq_load(j):
                return wqS[j * P:(j + 1) * P, :].rearrange(
                    "p (k f) -> p k f", k=KO)

            def xs_full(e):
                return xS[e * P:(e + 1) * P, :].rearrange(
                    "p (k t) -> p k t", k=KO)

            # slices 0,1 split across both rings (halves the wave-0 latency);
            # later slices whole, alternating; weights slotted before first
            # need per the measured ~0.17 MB/us per-ring arrival rate
            # the sync ring starts ~2us earlier and runs faster at the very
            # start, so the V/K-critical weights ride it first
            sync_q = [
                (wv_sb[:], wvS[:].rearrange("p (k f) -> p k f", k=KO)),
                (x_sb[0:64, 0], xs_half(0, 0)),
                (wk_sb[:], wkS[:].rearrange("p (k f) -> p k f", k=KO)),
                (x_sb[0:64, 1], xs_half(1, 0)),
                (wq_sb[:, 0], wq_load(0)),
                (wq_sb[:, 2], wq_load(2)),
                (x_sb[:, 2], xs_full(2)),
                (x_sb[:, 4], xs_full(4)),
                (x_sb[:, 6], xs_full(6)),
            ]
            scal_q = [
                (x_sb[64:128, 0], xs_half(0, 1)),
                (x_sb[64:128, 1], xs_half(1, 1)),
                (wq_sb[:, 1], wq_load(1)),
                (prm_sb[:], prmd[:]),
                (tri_sb[:], mskd[:]),
                (cos_sb[:], cosd[:]),
                (sin_sb[:], sind[:]),
                (wq_sb[:, 3], wq_load(3)),
                (x_sb[:, 3], xs_full(3)),
                (x_sb[:, 5], xs_full(5)),
                (x_sb[:, 7], xs_full(7)),
                (wo_sb[:], woS[:].rearrange("p (k d) -> p k d", k=4)),
            ]
            # both HWDGE rings (only sync + scalar have them on trn2); the
            # scalar engine just fires the triggers up-front, before any exp
            for dst, src in sync_q:
                nc.sync.dma_start(dst, src)
            for dst, src in scal_q:
                nc.scalar.dma_start(dst, src)

            def x_mv(nt, ko):
                """[128, 2, 256] moving view of x tokens [nt*512,(nt+1)*512)"""
                return x_sb[:, 2 * nt:2 * nt + 2, ko, :]

            # rope: dst = raw*cos + rotate_half(raw)*sin. The rotate-half is
            # a PE matmul against a constant 128x128 permutation matrix
            # (rot_ps = PRM.T @ raw, 213ns) — no DMA ring traffic, no scalar
            # engine. It reads the bf16 cast, so it is software-pipelined:
            # the perm matmul + combine of rope i are emitted at the start
            # of the NEXT unit (flush_rope), hiding the DVE-cast latency.
            rope_pending = []

            def flush_rope():
                while rope_pending:
                    rope_pending.pop(0)()

            def rope(dst, nt):
                ts = slice(nt * 512, (nt + 1) * 512)

                def fin(ps):
                    raw = work.tile([P, 512], BF16, tag="ropraw")
                    nc.vector.tensor_copy(raw[:], ps[:])

                    def finish():
                        rps = big_ps.tile([P, 512], F32, tag="big")
                        nc.tensor.matmul(rps[:], prm_sb[:], raw[:],
                                         start=True, stop=True)
                        t1 = work.tile([P, 512], BF16, tag="ropt1")
                        nc.vector.tensor_mul(t1[:], raw[:], cos_sb[:, ts])
                        rtb = work.tile([P, 512], BF16, tag="roprtb")
                        nc.vector.tensor_mul(rtb[:], rps[:], sin_sb[:, ts])
                        nc.vector.tensor_add(dst[:, ts], t1[:], rtb[:])
                    rope_pending.append(finish)
                return fin

            # ---- K projection + rope (feature-major K^T [128, T]) ----
            kt = const.tile([P, T], BF16, tag="kt")

            def k_proj(nt):
                flush_rope()
                ps = big_ps.tile([P, 512], F32, tag="big")
                for ko in range(KO):
                    nc.tensor.matmul(ps[:], wk_sb[:, ko, :], x_mv(nt, ko),
                                     start=(ko == 0), stop=(ko == KO - 1))
                rope(kt, nt)(ps)

            # ---- V projection (token-major, 64 ones columns per head) ----
            def v_proj(tt):
                flush_rope()
                ps = big_ps.tile([P, DKV], F32, tag="big")
                for ko in range(KO):
                    nc.tensor.matmul(
                        ps[:], x_sb[:, tt // 2, ko,
                                    (tt % 2) * P:(tt % 2) * P + P],
                        wv_sb[:, ko, :],
                        start=(ko == 0), stop=(ko == KO - 1))
                nc.vector.tensor_copy(v_sb[:, tt, 0:64], ps[:, 0:64])
                nc.vector.tensor_copy(v_sb[:, tt, 128:192], ps[:, 64:128])

            # ---- Q projection + rope for one head pair, one token tile ----
            qts = {}
            for j in range(4):
                qts[j] = const.tile([P, T], BF16, tag=f"qt{j}", name=f"qt{j}")

            def q_proj_nt(j, nt):
                flush_rope()
                ps = big_ps.tile([P, 512], F32, tag="big")
                for ko in range(KO):
                    nc.tensor.matmul(ps[:], wq_sb[:, j, ko, :], x_mv(nt, ko),
                                     start=(ko == 0), stop=(ko == KO - 1))
                rope(qts[j], nt)(ps)

            # ---- filler queue: PE work pulled between attention pairs.
            # Entries carry an estimated PE cost (us); pull(budget) drains
            # ~budget worth of work, carrying surplus credit so chunky units
            # (3.4us q-projections) average out over iterations ----
            pool = deque()
            pull_credit = [0.0]

            def pull(budget):
                pull_credit[0] += budget
                while pool and pull_credit[0] > 0:
                    cost, fn = pool.popleft()
                    fn()
                    pull_credit[0] -= cost

            def drain():
                while pool:
                    pool.popleft()[1]()
                pull_credit[0] = 0.0

            # ---- attention for one (qt, j) head-pair into ot tile ----
            def attn_block(qt, j, ot, split_epi=False):
                flush_rope()
                pv = pv_ps.tile([P, 1024], F32, tag="pv")
                nkb = 4 * qt + 4

                def flush_pv(prev):
                    # PV matmuls for the previous kb (software pipeline: issued
                    # after the next kb's scores so PE never waits on ACT's exp
                    # of the current block). Diagonal blocks only touch output
                    # columns >= their first causally-valid query. (Splitting
                    # each PV into two concurrent K=64 row-tiles crashes the
                    # device - two in-flight matmuls may not share a psum bank.)
                    pkb, c0, pp = prev
                    ppv = pp[:].rearrange("p (two t) -> p two t", two=2)
                    nc.tensor.matmul(pv[:, c0:512], v_sb[:, pkb, 0:128],
                                     ppv[:, 0, c0:512],
                                     start=(pkb == 0), stop=(pkb == nkb - 1))
                    nc.tensor.matmul(pv[:, 512 + c0:1024], v_sb[:, pkb, 128:256],
                                     ppv[:, 1, c0:512],
                                     start=(pkb == 0), stop=(pkb == nkb - 1))

                pending = []
                for kb0 in range(0, nkb, 2):
                    # issue TWO key-blocks' score pairs back-to-back: the
                    # later kt LDWEIGHTS pull ahead behind score matmuls in
                    # the other row group (a full-row PV matmul in between
                    # would block the pull-ahead and expose ~107ns each)
                    sps = []
                    for kb in (kb0, kb0 + 1):
                        tk = slice(kb * P, (kb + 1) * P)
                        jr = kb - 4 * qt       # >= 0 on diagonal blocks
                        c0 = max(0, jr) * P    # first causally-valid column
                        tqs = slice(qt * 512 + c0, (qt + 1) * 512)
                        # one 2-bank psum tile holds both heads' scores; the
                        # two matmuls write disjoint banks, then a SINGLE exp
                        # (3-dim AP) and a single broadcast mask cover both
                        # halves, halving the pacing-engine instruction count
                        sp = s_ps.tile([P, 1024], F32, tag="s")
                        spv = sp[:].rearrange("p (two t) -> p two t", two=2)
                        nc.tensor.matmul(sp[:, c0:512], kt[0:64, tk],
                                         qts[j][0:64, tqs],
                                         start=True, stop=True)
                        nc.tensor.matmul(sp[:, 512 + c0:1024], kt[64:128, tk],
                                         qts[j][64:128, tqs],
                                         start=True, stop=True)
                        sps.append((kb, c0, jr, spv))
                    while pending:
                        flush_pv(pending.pop(0))
                    for kb, c0, jr, spv in sps:
                        pp = pexp.tile([P, 1024], BF16, tag="p")
                        ppv = pp[:].rearrange("p (two t) -> p two t", two=2)
                        nc.scalar.activation(ppv[:, :, c0:512],
                                             spv[:, :, c0:512],
                                             AF.Exp, scale=SCALE)
                        if jr >= 0:
                            # triangle mask on the partially-valid block
                            nc.vector.tensor_mul(
                                ppv[:, :, c0:c0 + P], ppv[:, :, c0:c0 + P],
                                tri_sb[:, None, :].to_broadcast((P, 2, P)))
                        pending.append((kb, c0, pp))
                    pull(1.0)
                for pr in pending:
                    flush_pv(pr)
                # normalization: rows 64..127 of pv hold 64 copies of the
                # sum-exp row (ones trick): stage to SBUF, one wide
                # reciprocal, then fused multiplies finalize ot from PSUM
                den = rrp.tile([64, 1024], F32, tag="den")
                rec = rrp.tile([64, 1024], F32, tag="rec")
                if split_epi:
                    # per-head chain on the tail block: head0's normalize
                    # overlaps head1's final PV matmul
                    for hh in range(2):
                        cs = slice(hh * 512, hh * 512 + 512)
                        nc.vector.tensor_copy(den[:, cs], pv[64:128, cs])
                        nc.vector.reciprocal_approx_fast(rec[:, cs],
                                                         den[:, cs])
                        nc.vector.tensor_mul(ot[hh * 64:hh * 64 + 64, j, :],
                                             pv[0:64, cs], rec[:, cs])
                else:
                    nc.vector.tensor_copy(den[:], pv[64:128, :])
                    nc.vector.reciprocal_approx_fast(rec[:], den[:])
                    nc.vector.tensor_mul(ot[0:64, j, :], pv[0:64, 0:512],
                                         rec[:, 0:512])
                    nc.vector.tensor_mul(ot[64:128, j, :], pv[0:64, 512:1024],
                                         rec[:, 512:1024])

            # ---- Wo output projection, emitted as per-oc filler units ----
            ysbs = {}

            def wo_unit(qt, tt, oc, ot):
                flush_rope()
                r0 = qt * 512 + tt * P
                if oc == 0:
                    ysbs[(qt, tt)] = ysp.tile([P, D], BF16, tag="ysb",
                                              name=f"ysb{qt}_{tt}")
                ysb = ysbs[(qt, tt)]
                yps = big_ps.tile([P, 512], F32, tag="big")
                for kf in range(4):
                    nc.tensor.matmul(yps[:], ot[:, kf, tt * P:(tt + 1) * P],
                                     wo_sb[:, kf, oc * 512:(oc + 1) * 512],
                                     start=(kf == 0), stop=(kf == 3))
                nc.vector.tensor_copy(ysb[:, oc * 512:(oc + 1) * 512], yps[:])
                if oc == 3:
                    # one whole-row DMA (4KB descriptors) on the otherwise
                    # idle sync engine; scalar must stay free for exp
                    nc.sync.dma_start(y[r0:r0 + P, :], ysb[:])

            def queue_wo(qt):
                ot = ot_tiles[qt]
                for tt in range(4):
                    for oc in range(4):
                        pool.append((0.85,
                                     lambda qt=qt, tt=tt, oc=oc, ot=ot:
                                     wo_unit(qt, tt, oc, ot)))

            # ---- emission: dependency wavefront ----
            # wave 0 (x slices 0,1 + wk/wv/wq/cos/sin): V, K, all-j Q for nt0
            v_proj(0); v_proj(1); v_proj(2); v_proj(3)
            k_proj(0)
            for j in range(4):
                q_proj_nt(j, 0)

            ot_tiles = {qt: otp.tile([P, 4, 512], BF16, tag="ot",
                                     name=f"ot{qt}") for qt in range(4)}

            # region 1: attn qt0; filler = wave 1 (x slices 2,3)
            for f in [(0.9, lambda: v_proj(4)), (0.9, lambda: v_proj(5)),
                      (0.9, lambda: v_proj(6)), (0.9, lambda: v_proj(7)),
                      (3.4, lambda: k_proj(1)),
                      (3.4, lambda: q_proj_nt(0, 1)),
                      (3.4, lambda: q_proj_nt(1, 1)),
                      (3.4, lambda: q_proj_nt(2, 1)),
                      (3.4, lambda: q_proj_nt(3, 1))]:
                pool.append(f)
            for j in range(4):
                attn_block(0, j, ot_tiles[0])
                pull(1.0)
            drain()             # qt1 needs all of wave 1

            # region 2: attn qt1; filler = wave 3 (x slices 6,7), which must
            # fully precede qt3's attention, so it drains here
            for f in [(0.9, lambda: v_proj(12)), (0.9, lambda: v_proj(13)),
                      (0.9, lambda: v_proj(14)), (0.9, lambda: v_proj(15)),
                      (3.4, lambda: k_proj(3)),
                      (3.4, lambda: q_proj_nt(0, 3)),
                      (3.4, lambda: q_proj_nt(1, 3)),
                      (3.4, lambda: q_proj_nt(2, 3)),
                      (3.4, lambda: q_proj_nt(3, 3))]:
                pool.append(f)
            for j in range(4):
                attn_block(1, j, ot_tiles[1])
                pull(1.0)
            drain()

            # region 3: attn qt3 (heaviest exp load). Filler = wave 2 (only
            # needed by qt2, i.e. region 4) + Wo(qt0): enough PE work that
            # the scores never stall on the exp double-buffer.
            for f in [(0.9, lambda: v_proj(8)), (0.9, lambda: v_proj(9)),
                      (0.9, lambda: v_proj(10)), (0.9, lambda: v_proj(11)),
                      (3.4, lambda: k_proj(2)),
                      (3.4, lambda: q_proj_nt(0, 2)),
                      (3.4, lambda: q_proj_nt(1, 2)),
                      (3.4, lambda: q_proj_nt(2, 2)),
                      (3.4, lambda: q_proj_nt(3, 2))]:
                pool.append(f)
            queue_wo(0)
            for j in range(4):
                attn_block(3, j, ot_tiles[3])
                pull(1.0)
            drain()

            # region 4: attn qt2; filler = Wo(qt1) + Wo(qt3)
            queue_wo(1)
            queue_wo(3)
            for j in range(4):
                attn_block(2, j, ot_tiles[2],
                           split_epi=(j == 3))
                pull(1.0)
            drain()

            # tail: Wo(qt2)
            queue_wo(2)
            drain()

    nc.finalize()
    _nc_cache["nc"] = nc
    return nc


def make_in_maps(x, Wq, Wk, Wv, Wo):
    bf = ml_dtypes.bfloat16
    x = np.asarray(x, np.float32)
    Wq = np.asarray(Wq, np.float32)
    Wk = np.asarray(Wk, np.float32)
    Wv = np.asarray(Wv, np.float32)
    Wo = np.asarray(Wo, np.float32)

    # rope tables, [128, T]: row p covers head-dim d = p % 64
    half = HD // 2
    inv_freq = 1.0 / (ROPE_BASE ** (np.arange(half, dtype=np.float64) / half))
    pos = np.arange(T, dtype=np.float64)
    d_idx = np.arange(P) % HD
    freqs = pos[None, :] * inv_freq[d_idx % half][:, None]      # [128, T]
    cos_t = np.cos(freqs).astype(bf)
    sign = np.where(d_idx < half, -1.0, 1.0)[:, None]
    sin_t = (np.sin(freqs) * sign).astype(bf)

    # causal 0/1 triangle for the partially-valid diagonal sub-block
    pp = np.arange(P)[:, None]
    ff = np.arange(P)[None, :]
    tri = (ff >= pp).astype(bf)

    # rotate-half permutation matrix: prm[k, m] = 1 iff k == rot(m), so the
    # PE matmul prm.T @ raw yields raw[rot(m)] on partition m (sign lives in
    # the sin table)
    m_idx = np.arange(P)
    rot_m = np.where(m_idx % HD < half, m_idx + half, m_idx - half)
    prm = np.zeros((P, P), np.float32)
    prm[rot_m, m_idx] = 1.0
    prm = prm.astype(bf)

    def pack(a, n_chunks):
        # [n_chunks*128, F] -> [128, n_chunks*F] partition-major
        F = a.shape[1]
        return np.ascontiguousarray(
            a.reshape(n_chunks, P, F).transpose(1, 0, 2).reshape(P, n_chunks * F))

    in_maps = []
    for c in range(8):
        b, g = c // 4, c % 4
        heads = [8 * g + h for h in PERM_Q]
        qrows = np.concatenate([np.arange(h * HD, (h + 1) * HD) for h in heads])
        kvrows = np.arange(2 * g * HD, (2 * g + 2) * HD)
        xT = np.ascontiguousarray(x[b].T).astype(bf)             # [D, T]
        wqT = np.ascontiguousarray(Wq[qrows, :].T).astype(bf)    # [D, FQ]
        wkT = np.ascontiguousarray(Wk[kvrows, :].T).astype(bf)
        wvT = np.ascontiguousarray(Wv[kvrows, :].T).astype(bf)
        woT = np.ascontiguousarray(Wo[:, qrows].T).astype(bf)    # [FQ, D]
        # xS rows = e*128 + p, cols = ko*TE + t  (slice e, token e*TE+t)
        xs = np.ascontiguousarray(
            xT.reshape(KO, P, NXE, TE).transpose(2, 1, 0, 3)
            .reshape(NXE * P, KO * TE))
        # wqS rows = j*128 + p, cols = ko*128 + f
        wqs = np.ascontiguousarray(
            wqT.reshape(KO, P, 4, P).transpose(2, 1, 0, 3).reshape(4 * P, KO * P))
        in_maps.append({
            "xS": xs,
            "wqS": wqs,
            "wkS": pack(wkT, KO),
            "wvS": pack(wvT, KO),
            "woS": pack(woT, 4),
            "cosT": cos_t,
            "sinT": sin_t,
            "tri": tri,
            "prm": prm,
        })
    return in_maps


def combine_outputs(results):
    out = np.zeros((B, T, D), np.float32)
    for c in range(8):
        out[c // 4] += np.asarray(results[c]["y"], np.float32)
    return out


def _ensure_ntff_hook():
    """Register the axon NTFF profile hook (antenv.axon_hooks is missing
    from this image; recreate it and wire the ctypes hook from trn_boot)."""
    import sys, types
    if "antenv.axon_hooks" in sys.modules:
        return
    m = types.ModuleType("antenv.axon_hooks")
    hook = [None]
    m.set_axon_ntff_profile_hook = lambda h: hook.__setitem__(0, h)
    m.get_axon_ntff_profile_hook = lambda: hook[0]
    sys.modules["antenv.axon_hooks"] = m
    import antenv
    antenv.axon_hooks = m
    sys.path.insert(0, "/root/.axon_site")
    from trn_agent_boot.trn_boot import _ntff_profile_via_ctypes
    m.set_axon_ntff_profile_hook(
        _ntff_profile_via_ctypes("/opt/axon/libaxon_pjrt.so"))


def kernel(x, Wq, Wk, Wv, Wo, _trace=False):
    if _trace:
        _ensure_ntff_hook()
    nc = build_nc()
    in_maps = make_in_maps(x, Wq, Wk, Wv, Wo)
    res = run_bass_kernel_spmd(nc, in_maps, core_ids=list(range(8)), trace=_trace)
    out = combine_outputs(res.results)
    if _trace:
        return out, res
    return out


# revision 32
# speedup vs baseline: 1.1994x; 1.1994x over previous
"""Distributed GQA attention kernel for Trainium2 (8 NeuronCores).

Sharding: 2-way data parallel over batch x 4-way tensor parallel over heads.
Core c handles batch b = c // 4 and head group g = c % 4 (8 q-heads, 2 kv-heads).
Each core computes a full-size partial of the output (its head group pushed
through Wo); the host sums the 4 partials per batch. No on-device collective.

Device-side layout is feature-major (Q^T/K^T: [feature partitions, T free]) so
projections consume the host-pre-transposed x^T directly, attention scores are
computed transposed (S^T[tk, tq]) so softmax(P)@V needs no transposes, and the
softmax denominator is broadcast for free by 64 ones-columns appended to V
(the PV matmul then emits 64 identical sum-exp rows; normalization is a copy +
one [64,1024] reciprocal + fused multiplies out of PSUM; reciprocal must read
SBUF - the custom-DVE op returns garbage on a partition-offset PSUM source).
Score matmuls are K=64 so the two heads' matmuls land on different PE row
groups (base partitions 0/64) and stream CONCURRENTLY (2x packing).

Schedule: the kernel is PE-streaming-bound (~224us of matmul columns at
2.4GHz) with the scalar engine (exp: (N+352)/1.2ns per instruction, ~163us
total) second. The emission is a dependency wavefront: x arrives on the two
HWDGE rings (sync+scalar — the only engines with hardware DGE); ~80 dummy
matmuls warm the PE HAM clock-gate (cold=1.2GHz) while the first slices land;
V/K/Q projections chase the slices; attention for query-tile qt starts as
soon as its K/Q columns exist. Attention inner loops pull cost-metered
"filler" PE work (remaining projections, Wo output blocks) from a queue
between score/PV pairs so the PE never idles while ACT chews exp: the ACT
deficit is ~1us per 2-key-block iteration. ACT runs nothing but exp — rope's
psum cast runs on DVE and rotate-half is a PE matmul against a constant
128x128 permutation (software-pipelined behind the cast via flush_rope, so
the PE never waits on DVE). qt region order 0,1,3,2 balances each region's
exp load against available filler (wave3 projections must drain during qt1
since qt3 needs them; wave2 defers into the qt3 region as filler); Wo(qt2)
plus its output DMA is the only tail. Output y rides the idle sync engine
only — a dma_start occupies its issuing engine ~5ns/descriptor, which would
starve exp if placed on scalar.

Note: the chip randomly enters a ~2.0GHz power state (vs 2.4) for whole runs;
measured times swing ~15% run-to-run on identical code.
"""

import numpy as np
import ml_dtypes
from collections import deque
from contextlib import ExitStack

import concourse.bass as bass
from concourse import bacc
import concourse.mybir as mybir
import concourse.tile as tile
from concourse.bass_utils import run_bass_kernel_spmd

BF16 = mybir.dt.bfloat16
F32 = mybir.dt.float32
AF = mybir.ActivationFunctionType

P = 128
B, T, D = 2, 2048, 2048
NUM_HEADS, NUM_KV_HEADS, HD = 32, 8, 64
FQ = 512          # q features per core (8 heads x 64)
DKV = 128         # kv features per core (2 kv heads x 64)
KO = D // P       # 16 contraction tiles over d_model
NT = T // 512     # 4 tiles of 512 along T
NXE = 8           # x arrives in 8 T-slices of 256
TE = T // NXE
SCALE = 1.0 / np.sqrt(HD)
ROPE_BASE = 10000.0
# local head order inside the 512 q-features: pairs (j, j+4) so that the two
# heads in partition tile j sit at bases 0/64 matching kv heads 0/1 in K^T
PERM_Q = [0, 4, 1, 5, 2, 6, 3, 7]

_nc_cache = {}


def build_nc():
    if "nc" in _nc_cache:
        return _nc_cache["nc"]
    nc = bacc.Bacc()
    # host-packed layouts: row = slice*128 + partition, all loads contiguous
    xS = nc.declare_dram_parameter("xS", [NXE * P, KO * TE], BF16, isOutput=False)
    wqS = nc.declare_dram_parameter("wqS", [4 * P, KO * P], BF16, isOutput=False)
    wkS = nc.declare_dram_parameter("wkS", [P, KO * DKV], BF16, isOutput=False)
    wvS = nc.declare_dram_parameter("wvS", [P, KO * DKV], BF16, isOutput=False)
    woS = nc.declare_dram_parameter("woS", [P, 4 * D], BF16, isOutput=False)
    cosd = nc.declare_dram_parameter("cosT", [P, T], BF16, isOutput=False)
    sind = nc.declare_dram_parameter("sinT", [P, T], BF16, isOutput=False)
    mskd = nc.declare_dram_parameter("tri", [P, P], BF16, isOutput=False)
    prmd = nc.declare_dram_parameter("prm", [P, P], BF16, isOutput=False)
    y = nc.declare_dram_parameter("y", [T, D], BF16, isOutput=True)

    with tile.TileContext(nc) as tc:
        with ExitStack() as ctx:
            const = ctx.enter_context(tc.tile_pool(name="const", bufs=1))
            work = ctx.enter_context(tc.tile_pool(name="work", bufs=6))
            otp = ctx.enter_context(tc.tile_pool(name="otp", bufs=2))
            pexp = ctx.enter_context(tc.tile_pool(name="pexp", bufs=8))
            rrp = ctx.enter_context(tc.tile_pool(name="rrp", bufs=2))
            ysp = ctx.enter_context(tc.tile_pool(name="ysp", bufs=2))
            big_ps = ctx.enter_context(tc.tile_pool(name="bigps", bufs=2, space="PSUM"))
            pv_ps = ctx.enter_context(tc.tile_pool(name="pvps", bufs=1, space="PSUM"))
            s_ps = ctx.enter_context(tc.tile_pool(name="sps", bufs=2, space="PSUM"))

            x_sb = const.tile([P, NXE, KO, TE], BF16, tag="x")
            wq_sb = const.tile([P, 4, KO, P], BF16, tag="wq")
            wv_sb = const.tile([P, KO, DKV], BF16, tag="wv")
            wk_sb = const.tile([P, KO, DKV], BF16, tag="wk")
            cos_sb = const.tile([P, T], BF16, tag="cos")
            sin_sb = const.tile([P, T], BF16, tag="sin")
            tri_sb = const.tile([P, P], BF16, tag="tri")
            prm_sb = const.tile([P, P], BF16, tag="prm")
            wo_sb = const.tile([P, 4, D], BF16, tag="wo")
            warm_sb = const.tile([P, P], BF16, tag="warm")

            # ---- V layout + PE warm-up (emitted first: gpsimd memsets, then
            # dummy matmuls keep the PE busy from ~6.4us so the HAM clock-gate
            # reaches 8/8 before real data lands, and stays there) ----
            v_sb = const.tile([P, 16, 256], BF16, tag="v")
            nc.gpsimd.memset(warm_sb[:], 0.0)
            nc.gpsimd.memset(v_sb[:, :, 64:128], 1.0)
            nc.gpsimd.memset(v_sb[:, :, 192:256], 1.0)
            wps = big_ps.tile([P, P], F32, tag="big")
            for _ in range(82):
                nc.tensor.matmul(wps[:], warm_sb[:], warm_sb[:],
                                 start=True, stop=True)

            # ---- input loads: two HWDGE rings (sync + gpsimd), x slices
            # split into partition halves so both rings carry every slice;
            # weights slotted just before first need ----
            def xs_half(e, h):
                return xS[e * P + 64 * h:e * P + 64 * (h + 1), :].rearrange(
                    "p (k t) -> p k t", k=KO)

            def wq_load(j):
                return wqS[j * P:(j + 1) * P, :].rearrange(
                    "p (k f) -> p k f", k=KO)

            def xs_full(e):
                return xS[e * P:(e + 1) * P, :].rearrange(
                    "p (k t) -> p k t", k=KO)

            # slices 0,1 split across both rings (halves the wave-0 latency);
            # later slices whole, alternating; weights slotted before first
            # need per the measured ~0.17 MB/us per-ring arrival rate
            def xs_q(e, q):
                return xS[e * P + 32 * q:e * P + 32 * (q + 1), :].rearrange(
                    "p (k t) -> p k t", k=KO)

            # slice 0 split into quarters across both rings so the first V
            # matmul can start ~14.4us instead of ~19; V/K-critical weights
            # ride the sync ring (it starts ~2us earlier)
            sync_q = [
                (x_sb[0:32, 0], xs_q(0, 0)),
                (x_sb[32:64, 0], xs_q(0, 1)),
                (wv_sb[:], wvS[:].rearrange("p (k f) -> p k f", k=KO)),
                (x_sb[0:64, 1], xs_half(1, 0)),
                (wk_sb[:], wkS[:].rearrange("p (k f) -> p k f", k=KO)),
                (wq_sb[:, 0], wq_load(0)),
                (wq_sb[:, 2], wq_load(2)),
                (x_sb[:, 2], xs_full(2)),
                (x_sb[:, 4], xs_full(4)),
                (x_sb[:, 6], xs_full(6)),
            ]
            scal_q = [
                (x_sb[64:96, 0], xs_q(0, 2)),
                (x_sb[96:128, 0], xs_q(0, 3)),
                (x_sb[64:128, 1], xs_half(1, 1)),
                (wq_sb[:, 1], wq_load(1)),
                (prm_sb[:], prmd[:]),
                (tri_sb[:], mskd[:]),
                (cos_sb[:], cosd[:]),
                (sin_sb[:], sind[:]),
                (wq_sb[:, 3], wq_load(3)),
                (x_sb[:, 3], xs_full(3)),
                (x_sb[:, 5], xs_full(5)),
                (x_sb[:, 7], xs_full(7)),
                (wo_sb[:], woS[:].rearrange("p (k d) -> p k d", k=4)),
            ]
            # both HWDGE rings (only sync + scalar have them on trn2); the
            # scalar engine just fires the triggers up-front, before any exp
            for dst, src in sync_q:
                nc.sync.dma_start(dst, src)
            for dst, src in scal_q:
                nc.scalar.dma_start(dst, src)

            def x_mv(nt, ko):
                """[128, 2, 256] moving view of x tokens [nt*512,(nt+1)*512)"""
                return x_sb[:, 2 * nt:2 * nt + 2, ko, :]

            # rope: dst = raw*cos + rotate_half(raw)*sin. The rotate-half is
            # a PE matmul against a constant 128x128 permutation matrix
            # (rot_ps = PRM.T @ raw, 213ns) — no DMA ring traffic, no scalar
            # engine. It reads the bf16 cast, so it is software-pipelined:
            # the perm matmul + combine of rope i are emitted at the start
            # of the NEXT unit (flush_rope), hiding the DVE-cast latency.
            rope_pending = []

            def flush_rope():
                while rope_pending:
                    rope_pending.pop(0)()

            def rope(dst, nt):
                ts = slice(nt * 512, (nt + 1) * 512)

                def fin(ps):
                    raw = work.tile([P, 512], BF16, tag="ropraw")
                    nc.vector.tensor_copy(raw[:], ps[:])

                    def finish():
                        rps = big_ps.tile([P, 512], F32, tag="big")
                        nc.tensor.matmul(rps[:], prm_sb[:], raw[:],
                                         start=True, stop=True)
                        t1 = work.tile([P, 512], BF16, tag="ropt1")
                        nc.vector.tensor_mul(t1[:], raw[:], cos_sb[:, ts])
                        rtb = work.tile([P, 512], BF16, tag="roprtb")
                        nc.vector.tensor_mul(rtb[:], rps[:], sin_sb[:, ts])
                        nc.vector.tensor_add(dst[:, ts], t1[:], rtb[:])
                    rope_pending.append(finish)
                return fin

            # ---- K projection + rope (feature-major K^T [128, T]) ----
            kt = const.tile([P, T], BF16, tag="kt")

            def k_proj(nt):
                flush_rope()
                ps = big_ps.tile([P, 512], F32, tag="big")
                for ko in range(KO):
                    nc.tensor.matmul(ps[:], wk_sb[:, ko, :], x_mv(nt, ko),
                                     start=(ko == 0), stop=(ko == KO - 1))
                rope(kt, nt)(ps)

            # ---- V projection (token-major, 64 ones columns per head) ----
            def v_proj(tt):
                flush_rope()
                ps = big_ps.tile([P, DKV], F32, tag="big")
                for ko in range(KO):
                    nc.tensor.matmul(
                        ps[:], x_sb[:, tt // 2, ko,
                                    (tt % 2) * P:(tt % 2) * P + P],
                        wv_sb[:, ko, :],
                        start=(ko == 0), stop=(ko == KO - 1))
                nc.vector.tensor_copy(v_sb[:, tt, 0:64], ps[:, 0:64])
                nc.vector.tensor_copy(v_sb[:, tt, 128:192], ps[:, 64:128])

            # ---- Q projection + rope for one head pair, one token tile ----
            qts = {}
            for j in range(4):
                qts[j] = const.tile([P, T], BF16, tag=f"qt{j}", name=f"qt{j}")

            def q_proj_nt(j, nt):
                flush_rope()
                ps = big_ps.tile([P, 512], F32, tag="big")
                for ko in range(KO):
                    nc.tensor.matmul(ps[:], wq_sb[:, j, ko, :], x_mv(nt, ko),
                                     start=(ko == 0), stop=(ko == KO - 1))
                rope(qts[j], nt)(ps)

            # ---- filler queue: PE work pulled between attention pairs.
            # Entries carry an estimated PE cost (us); pull(budget) drains
            # ~budget worth of work, carrying surplus credit so chunky units
            # (3.4us q-projections) average out over iterations ----
            pool = deque()
            pull_credit = [0.0]

            def pull(budget):
                pull_credit[0] += budget
                while pool and pull_credit[0] > 0:
                    cost, fn = pool.popleft()
                    fn()
                    pull_credit[0] -= cost

            def drain():
                while pool:
                    pool.popleft()[1]()
                pull_credit[0] = 0.0

            # ---- attention for one (qt, j) head-pair into ot tile ----
            def attn_block(qt, j, ot, split_epi=False):
                flush_rope()
                pv = pv_ps.tile([P, 1024], F32, tag="pv")
                nkb = 4 * qt + 4

                def flush_pv(prev):
                    # PV matmuls for the previous kb (software pipeline: issued
                    # after the next kb's scores so PE never waits on ACT's exp
                    # of the current block). Diagonal blocks only touch output
                    # columns >= their first causally-valid query. (Splitting
                    # each PV into two concurrent K=64 row-tiles crashes the
                    # device - two in-flight matmuls may not share a psum bank.)
                    pkb, c0, pp = prev
                    ppv = pp[:].rearrange("p (two t) -> p two t", two=2)
                    nc.tensor.matmul(pv[:, c0:512], v_sb[:, pkb, 0:128],
                                     ppv[:, 0, c0:512],
                                     start=(pkb == 0), stop=(pkb == nkb - 1))
                    nc.tensor.matmul(pv[:, 512 + c0:1024], v_sb[:, pkb, 128:256],
                                     ppv[:, 1, c0:512],
                                     start=(pkb == 0), stop=(pkb == nkb - 1))

                pending = []
                for kb0 in range(0, nkb, 2):
                    # issue TWO key-blocks' score pairs back-to-back: the
                    # later kt LDWEIGHTS pull ahead behind score matmuls in
                    # the other row group (a full-row PV matmul in between
                    # would block the pull-ahead and expose ~107ns each)
                    sps = []
                    for kb in (kb0, kb0 + 1):
                        tk = slice(kb * P, (kb + 1) * P)
                        jr = kb - 4 * qt       # >= 0 on diagonal blocks
                        c0 = max(0, jr) * P    # first causally-valid column
                        tqs = slice(qt * 512 + c0, (qt + 1) * 512)
                        # one 2-bank psum tile holds both heads' scores; the
                        # two matmuls write disjoint banks, then a SINGLE exp
                        # (3-dim AP) and a single broadcast mask cover both
                        # halves, halving the pacing-engine instruction count
                        sp = s_ps.tile([P, 1024], F32, tag="s")
                        spv = sp[:].rearrange("p (two t) -> p two t", two=2)
                        nc.tensor.matmul(sp[:, c0:512], kt[0:64, tk],
                                         qts[j][0:64, tqs],
                                         start=True, stop=True)
                        nc.tensor.matmul(sp[:, 512 + c0:1024], kt[64:128, tk],
                                         qts[j][64:128, tqs],
                                         start=True, stop=True)
                        sps.append((kb, c0, jr, spv))
                    while pending:
                        flush_pv(pending.pop(0))
                    for kb, c0, jr, spv in sps:
                        pp = pexp.tile([P, 1024], BF16, tag="p")
                        ppv = pp[:].rearrange("p (two t) -> p two t", two=2)
                        nc.scalar.activation(ppv[:, :, c0:512],
                                             spv[:, :, c0:512],
                                             AF.Exp, scale=SCALE)
                        if jr >= 0:
                            # triangle mask on the partially-valid block
                            nc.vector.tensor_mul(
                                ppv[:, :, c0:c0 + P], ppv[:, :, c0:c0 + P],
                                tri_sb[:, None, :].to_broadcast((P, 2, P)))
                        pending.append((kb, c0, pp))
                    pull(1.0)
                for pr in pending:
                    flush_pv(pr)
                # normalization: rows 64..127 of pv hold 64 copies of the
                # sum-exp row (ones trick): stage to SBUF, one wide
                # reciprocal, then fused multiplies finalize ot from PSUM
                den = rrp.tile([64, 1024], F32, tag="den")
                rec = rrp.tile([64, 1024], F32, tag="rec")
                if split_epi:
                    # per-head chain on the tail block: head0's normalize
                    # overlaps head1's final PV matmul
                    for hh in range(2):
                        cs = slice(hh * 512, hh * 512 + 512)
                        nc.vector.tensor_copy(den[:, cs], pv[64:128, cs])
                        nc.vector.reciprocal_approx_fast(rec[:, cs],
                                                         den[:, cs])
                        nc.vector.tensor_mul(ot[hh * 64:hh * 64 + 64, j, :],
                                             pv[0:64, cs], rec[:, cs])
                else:
                    nc.vector.tensor_copy(den[:], pv[64:128, :])
                    nc.vector.reciprocal_approx_fast(rec[:], den[:])
                    nc.vector.tensor_mul(ot[0:64, j, :], pv[0:64, 0:512],
                                         rec[:, 0:512])
                    nc.vector.tensor_mul(ot[64:128, j, :], pv[0:64, 512:1024],
                                         rec[:, 512:1024])

            # ---- Wo output projection, emitted as per-oc filler units ----
            ysbs = {}

            def wo_unit(qt, tt, oc, ot):
                flush_rope()
                r0 = qt * 512 + tt * P
                if oc == 0:
                    ysbs[(qt, tt)] = ysp.tile([P, D], BF16, tag="ysb",
                                              name=f"ysb{qt}_{tt}")
                ysb = ysbs[(qt, tt)]
                yps = big_ps.tile([P, 512], F32, tag="big")
                for kf in range(4):
                    nc.tensor.matmul(yps[:], ot[:, kf, tt * P:(tt + 1) * P],
                                     wo_sb[:, kf, oc * 512:(oc + 1) * 512],
                                     start=(kf == 0), stop=(kf == 3))
                nc.vector.tensor_copy(ysb[:, oc * 512:(oc + 1) * 512], yps[:])
                if qt == 2:
                    # tail blocks: ship each oc chunk immediately so the
                    # final drain is one 128KB chunk, not a 0.5MB row
                    nc.sync.dma_start(y[r0:r0 + P, oc * 512:(oc + 1) * 512],
                                      ysb[:, oc * 512:(oc + 1) * 512])
                elif oc == 3:
                    # one whole-row DMA (4KB descriptors) on the otherwise
                    # idle sync engine; scalar must stay free for exp
                    nc.sync.dma_start(y[r0:r0 + P, :], ysb[:])

            def queue_wo(qt):
                ot = ot_tiles[qt]
                for tt in range(4):
                    for oc in range(4):
                        pool.append((0.85,
                                     lambda qt=qt, tt=tt, oc=oc, ot=ot:
                                     wo_unit(qt, tt, oc, ot)))

            # ---- emission: dependency wavefront ----
            # wave 0 (x slices 0,1 + wk/wv/wq/cos/sin): V, K, all-j Q for nt0
            v_proj(0); v_proj(1); v_proj(2); v_proj(3)
            k_proj(0)
            for j in range(4):
                q_proj_nt(j, 0)

            ot_tiles = {qt: otp.tile([P, 4, 512], BF16, tag="ot",
                                     name=f"ot{qt}") for qt in range(4)}

            # region 1: attn qt0; filler = wave 1 (x slices 2,3)
            for f in [(0.9, lambda: v_proj(4)), (0.9, lambda: v_proj(5)),
                      (0.9, lambda: v_proj(6)), (0.9, lambda: v_proj(7)),
                      (3.4, lambda: k_proj(1)),
                      (3.4, lambda: q_proj_nt(0, 1)),
                      (3.4, lambda: q_proj_nt(1, 1)),
                      (3.4, lambda: q_proj_nt(2, 1)),
                      (3.4, lambda: q_proj_nt(3, 1))]:
                pool.append(f)
            for j in range(4):
                attn_block(0, j, ot_tiles[0])
                pull(1.0)
            drain()             # qt1 needs all of wave 1

            # region 2: attn qt1; filler = wave 3 (x slices 6,7), which must
            # fully precede qt3's attention, so it drains here
            for f in [(0.9, lambda: v_proj(12)), (0.9, lambda: v_proj(13)),
                      (0.9, lambda: v_proj(14)), (0.9, lambda: v_proj(15)),
                      (3.4, lambda: k_proj(3)),
                      (3.4, lambda: q_proj_nt(0, 3)),
                      (3.4, lambda: q_proj_nt(1, 3)),
                      (3.4, lambda: q_proj_nt(2, 3)),
                      (3.4, lambda: q_proj_nt(3, 3))]:
                pool.append(f)
            for j in range(4):
                attn_block(1, j, ot_tiles[1])
                pull(1.0)
            drain()

            # region 3: attn qt3 (heaviest exp load). Filler = wave 2 (only
            # needed by qt2, i.e. region 4) + Wo(qt0): enough PE work that
            # the scores never stall on the exp double-buffer.
            for f in [(0.9, lambda: v_proj(8)), (0.9, lambda: v_proj(9)),
                      (0.9, lambda: v_proj(10)), (0.9, lambda: v_proj(11)),
                      (3.4, lambda: k_proj(2)),
                      (3.4, lambda: q_proj_nt(0, 2)),
                      (3.4, lambda: q_proj_nt(1, 2)),
                      (3.4, lambda: q_proj_nt(2, 2)),
                      (3.4, lambda: q_proj_nt(3, 2))]:
                pool.append(f)
            queue_wo(0)
            for j in range(4):
                attn_block(3, j, ot_tiles[3])
                pull(1.0)
            drain()

            # region 4: attn qt2; filler = Wo(qt1) + Wo(qt3)
            queue_wo(1)
            queue_wo(3)
            for j in range(4):
                attn_block(2, j, ot_tiles[2],
                           split_epi=(j == 3))
                pull(1.0)
            drain()

            # tail: Wo(qt2)
            queue_wo(2)
            drain()

    nc.finalize()
    _nc_cache["nc"] = nc
    return nc


def make_in_maps(x, Wq, Wk, Wv, Wo):
    bf = ml_dtypes.bfloat16
    x = np.asarray(x, np.float32)
    Wq = np.asarray(Wq, np.float32)
    Wk = np.asarray(Wk, np.float32)
    Wv = np.asarray(Wv, np.float32)
    Wo = np.asarray(Wo, np.float32)

    # rope tables, [128, T]: row p covers head-dim d = p % 64
    half = HD // 2
    inv_freq = 1.0 / (ROPE_BASE ** (np.arange(half, dtype=np.float64) / half))
    pos = np.arange(T, dtype=np.float64)
    d_idx = np.arange(P) % HD
    freqs = pos[None, :] * inv_freq[d_idx % half][:, None]      # [128, T]
    cos_t = np.cos(freqs).astype(bf)
    sign = np.where(d_idx < half, -1.0, 1.0)[:, None]
    sin_t = (np.sin(freqs) * sign).astype(bf)

    # causal 0/1 triangle for the partially-valid diagonal sub-block
    pp = np.arange(P)[:, None]
    ff = np.arange(P)[None, :]
    tri = (ff >= pp).astype(bf)

    # rotate-half permutation matrix: prm[k, m] = 1 iff k == rot(m), so the
    # PE matmul prm.T @ raw yields raw[rot(m)] on partition m (sign lives in
    # the sin table)
    m_idx = np.arange(P)
    rot_m = np.where(m_idx % HD < half, m_idx + half, m_idx - half)
    prm = np.zeros((P, P), np.float32)
    prm[rot_m, m_idx] = 1.0
    prm = prm.astype(bf)

    def pack(a, n_chunks):
        # [n_chunks*128, F] -> [128, n_chunks*F] partition-major
        F = a.shape[1]
        return np.ascontiguousarray(
            a.reshape(n_chunks, P, F).transpose(1, 0, 2).reshape(P, n_chunks * F))

    in_maps = []
    for c in range(8):
        b, g = c // 4, c % 4
        heads = [8 * g + h for h in PERM_Q]
        qrows = np.concatenate([np.arange(h * HD, (h + 1) * HD) for h in heads])
        kvrows = np.arange(2 * g * HD, (2 * g + 2) * HD)
        xT = np.ascontiguousarray(x[b].T).astype(bf)             # [D, T]
        wqT = np.ascontiguousarray(Wq[qrows, :].T).astype(bf)    # [D, FQ]
        wkT = np.ascontiguousarray(Wk[kvrows, :].T).astype(bf)
        wvT = np.ascontiguousarray(Wv[kvrows, :].T).astype(bf)
        woT = np.ascontiguousarray(Wo[:, qrows].T).astype(bf)    # [FQ, D]
        # xS rows = e*128 + p, cols = ko*TE + t  (slice e, token e*TE+t)
        xs = np.ascontiguousarray(
            xT.reshape(KO, P, NXE, TE).transpose(2, 1, 0, 3)
            .reshape(NXE * P, KO * TE))
        # wqS rows = j*128 + p, cols = ko*128 + f
        wqs = np.ascontiguousarray(
            wqT.reshape(KO, P, 4, P).transpose(2, 1, 0, 3).reshape(4 * P, KO * P))
        in_maps.append({
            "xS": xs,
            "wqS": wqs,
            "wkS": pack(wkT, KO),
            "wvS": pack(wvT, KO),
            "woS": pack(woT, 4),
            "cosT": cos_t,
            "sinT": sin_t,
            "tri": tri,
            "prm": prm,
        })
    return in_maps


def combine_outputs(results):
    out = np.zeros((B, T, D), np.float32)
    for c in range(8):
        out[c // 4] += np.asarray(results[c]["y"], np.float32)
    return out


def _ensure_ntff_hook():
    """Register the axon NTFF profile hook (antenv.axon_hooks is missing
    from this image; recreate it and wire the ctypes hook from trn_boot)."""
    import sys, types
    if "antenv.axon_hooks" in sys.modules:
        return
    m = types.ModuleType("antenv.axon_hooks")
    hook = [None]
    m.set_axon_ntff_profile_hook = lambda h: hook.__setitem__(0, h)
    m.get_axon_ntff_profile_hook = lambda: hook[0]
    sys.modules["antenv.axon_hooks"] = m
    import antenv
    antenv.axon_hooks = m
    sys.path.insert(0, "/root/.axon_site")
    from trn_agent_boot.trn_boot import _ntff_profile_via_ctypes
    m.set_axon_ntff_profile_hook(
        _ntff_profile_via_ctypes("/opt/axon/libaxon_pjrt.so"))


def kernel(x, Wq, Wk, Wv, Wo, _trace=False):
    if _trace:
        _ensure_ntff_hook()
    nc = build_nc()
    in_maps = make_in_maps(x, Wq, Wk, Wv, Wo)
    res = run_bass_kernel_spmd(nc, in_maps, core_ids=list(range(8)), trace=_trace)
    out = combine_outputs(res.results)
    if _trace:
        return out, res
    return out


# revision 35
# speedup vs baseline: 1.2069x; 1.0063x over previous
"""Distributed GQA attention kernel for Trainium2 (8 NeuronCores).

Sharding: 2-way data parallel over batch x 4-way tensor parallel over heads.
Core c handles batch b = c // 4 and head group g = c % 4 (8 q-heads, 2 kv-heads).
Each core computes a full-size partial of the output (its head group pushed
through Wo); the host sums the 4 partials per batch. No on-device collective.

Device-side layout is feature-major (Q^T/K^T: [feature partitions, T free]) so
projections consume the host-pre-transposed x^T directly, attention scores are
computed transposed (S^T[tk, tq]) so softmax(P)@V needs no transposes, and the
softmax denominator is broadcast for free by 64 ones-columns appended to V
(the PV matmul then emits 64 identical sum-exp rows; normalization is a copy +
one [64,1024] reciprocal + fused multiplies out of PSUM; reciprocal must read
SBUF - the custom-DVE op returns garbage on a partition-offset PSUM source).
Score matmuls are K=64 so the two heads' matmuls land on different PE row
groups (base partitions 0/64) and stream CONCURRENTLY (2x packing).

Schedule: the kernel is PE-streaming-bound (~224us of matmul columns at
2.4GHz) with the scalar engine (exp: (N+352)/1.2ns per instruction, ~163us
total) second. The emission is a dependency wavefront: x arrives on the two
HWDGE rings (sync+scalar — the only engines with hardware DGE); ~80 dummy
matmuls warm the PE HAM clock-gate (cold=1.2GHz) while the first slices land;
V/K/Q projections chase the slices; attention for query-tile qt starts as
soon as its K/Q columns exist. Attention inner loops pull cost-metered
"filler" PE work (remaining projections, Wo output blocks) from a queue
between score/PV pairs so the PE never idles while ACT chews exp: the ACT
deficit is ~1us per 2-key-block iteration. ACT runs nothing but exp — rope's
psum cast runs on DVE and rotate-half is a PE matmul against a constant
128x128 permutation (software-pipelined behind the cast via flush_rope, so
the PE never waits on DVE). qt region order 0,1,3,2 balances each region's
exp load against available filler (wave3 projections must drain during qt1
since qt3 needs them; wave2 defers into the qt3 region as filler); Wo(qt2)
plus its output DMA is the only tail. Output y rides the idle sync engine
only — a dma_start occupies its issuing engine ~5ns/descriptor, which would
starve exp if placed on scalar.

Note: the chip randomly enters a ~2.0GHz power state (vs 2.4) for whole runs;
measured times swing ~15% run-to-run on identical code.
"""

import numpy as np
import ml_dtypes
from collections import deque
from contextlib import ExitStack

import concourse.bass as bass
from concourse import bacc
import concourse.mybir as mybir
import concourse.tile as tile
from concourse.bass_utils import run_bass_kernel_spmd

BF16 = mybir.dt.bfloat16
F32 = mybir.dt.float32
AF = mybir.ActivationFunctionType

P = 128
B, T, D = 2, 2048, 2048
NUM_HEADS, NUM_KV_HEADS, HD = 32, 8, 64
FQ = 512          # q features per core (8 heads x 64)
DKV = 128         # kv features per core (2 kv heads x 64)
KO = D // P       # 16 contraction tiles over d_model
NT = T // 512     # 4 tiles of 512 along T
NXE = 8           # x arrives in 8 T-slices of 256
TE = T // NXE
SCALE = 1.0 / np.sqrt(HD)
ROPE_BASE = 10000.0
# local head order inside the 512 q-features: pairs (j, j+4) so that the two
# heads in partition tile j sit at bases 0/64 matching kv heads 0/1 in K^T
PERM_Q = [0, 4, 1, 5, 2, 6, 3, 7]

_nc_cache = {}


def build_nc():
    if "nc" in _nc_cache:
        return _nc_cache["nc"]
    nc = bacc.Bacc()
    # host-packed layouts: row = slice*128 + partition, all loads contiguous
    xS = nc.declare_dram_parameter("xS", [NXE * P, KO * TE], BF16, isOutput=False)
    wqS = nc.declare_dram_parameter("wqS", [4 * P, KO * P], BF16, isOutput=False)
    wkS = nc.declare_dram_parameter("wkS", [P, KO * DKV], BF16, isOutput=False)
    wvS = nc.declare_dram_parameter("wvS", [P, KO * DKV], BF16, isOutput=False)
    woS = nc.declare_dram_parameter("woS", [P, 4 * D], BF16, isOutput=False)
    cosd = nc.declare_dram_parameter("cosT", [P, T], BF16, isOutput=False)
    sind = nc.declare_dram_parameter("sinT", [P, T], BF16, isOutput=False)
    mskd = nc.declare_dram_parameter("tri", [P, P], BF16, isOutput=False)
    prmd = nc.declare_dram_parameter("prm", [P, P], BF16, isOutput=False)
    y = nc.declare_dram_parameter("y", [T, D], BF16, isOutput=True)

    with tile.TileContext(nc) as tc:
        with ExitStack() as ctx:
            const = ctx.enter_context(tc.tile_pool(name="const", bufs=1))
            work = ctx.enter_context(tc.tile_pool(name="work", bufs=6))
            otp = ctx.enter_context(tc.tile_pool(name="otp", bufs=2))
            pexp = ctx.enter_context(tc.tile_pool(name="pexp", bufs=8))
            rrp = ctx.enter_context(tc.tile_pool(name="rrp", bufs=2))
            ysp = ctx.enter_context(tc.tile_pool(name="ysp", bufs=2))
            big_ps = ctx.enter_context(tc.tile_pool(name="bigps", bufs=2, space="PSUM"))
            pv_ps = ctx.enter_context(tc.tile_pool(name="pvps", bufs=1, space="PSUM"))
            s_ps = ctx.enter_context(tc.tile_pool(name="sps", bufs=2, space="PSUM"))

            x_sb = const.tile([P, NXE, KO, TE], BF16, tag="x")
            wq_sb = const.tile([P, 4, KO, P], BF16, tag="wq")
            wv_sb = const.tile([P, KO, DKV], BF16, tag="wv")
            wk_sb = const.tile([P, KO, DKV], BF16, tag="wk")
            cos_sb = const.tile([P, T], BF16, tag="cos")
            sin_sb = const.tile([P, T], BF16, tag="sin")
            tri_sb = const.tile([P, P], BF16, tag="tri")
            prm_sb = const.tile([P, P], BF16, tag="prm")
            wo_sb = const.tile([P, 4, D], BF16, tag="wo")
            warm_sb = const.tile([P, P], BF16, tag="warm")

            # ---- V layout + PE warm-up (emitted first: gpsimd memsets, then
            # dummy matmuls keep the PE busy from ~6.4us so the HAM clock-gate
            # reaches 8/8 before real data lands, and stays there) ----
            v_sb = const.tile([P, 16, 256], BF16, tag="v")
            nc.gpsimd.memset(warm_sb[:], 0.0)
            nc.gpsimd.memset(v_sb[:, :, 64:128], 1.0)
            nc.gpsimd.memset(v_sb[:, :, 192:256], 1.0)
            wps = big_ps.tile([P, P], F32, tag="big")
            for _ in range(82):
                nc.tensor.matmul(wps[:], warm_sb[:], warm_sb[:],
                                 start=True, stop=True)

            # ---- input loads: two HWDGE rings (sync + gpsimd), x slices
            # split into partition halves so both rings carry every slice;
            # weights slotted just before first need ----
            def xs_half(e, h):
                return xS[e * P + 64 * h:e * P + 64 * (h + 1), :].rearrange(
                    "p (k t) -> p k t", k=KO)

            def wq_load(j):
                return wqS[j * P:(j + 1) * P, :].rearrange(
                    "p (k f) -> p k f", k=KO)

            def xs_full(e):
                return xS[e * P:(e + 1) * P, :].rearrange(
                    "p (k t) -> p k t", k=KO)

            # slices 0,1 split across both rings (halves the wave-0 latency);
            # later slices whole, alternating; weights slotted before first
            # need per the measured ~0.17 MB/us per-ring arrival rate
            def xs_q(e, q):
                return xS[e * P + 32 * q:e * P + 32 * (q + 1), :].rearrange(
                    "p (k t) -> p k t", k=KO)

            # slice 0 split into quarters across both rings so the first V
            # matmul can start ~14.4us instead of ~19; V/K-critical weights
            # ride the sync ring (it starts ~2us earlier)
            wv_src = wvS[:].rearrange("p (k f) -> p k f", k=KO)
            sync_q = [
                (x_sb[0:32, 0], xs_q(0, 0)),
                (x_sb[32:64, 0], xs_q(0, 1)),
                (wv_sb[0:64], wv_src[0:64]),
                (x_sb[0:64, 1], xs_half(1, 0)),
                (wk_sb[:], wkS[:].rearrange("p (k f) -> p k f", k=KO)),
                (wq_sb[:, 0], wq_load(0)),
                (wq_sb[:, 2], wq_load(2)),
                (x_sb[:, 2], xs_full(2)),
                (x_sb[:, 4], xs_full(4)),
                (x_sb[:, 6], xs_full(6)),
            ]
            scal_q = [
                (x_sb[64:96, 0], xs_q(0, 2)),
                (x_sb[96:128, 0], xs_q(0, 3)),
                (wv_sb[64:128], wv_src[64:128]),
                (x_sb[64:128, 1], xs_half(1, 1)),
                (wq_sb[:, 1], wq_load(1)),
                (prm_sb[:], prmd[:]),
                (tri_sb[:], mskd[:]),
                (cos_sb[:], cosd[:]),
                (sin_sb[:], sind[:]),
                (wq_sb[:, 3], wq_load(3)),
                (x_sb[:, 3], xs_full(3)),
                (x_sb[:, 5], xs_full(5)),
                (x_sb[:, 7], xs_full(7)),
                (wo_sb[:], woS[:].rearrange("p (k d) -> p k d", k=4)),
            ]
            # both HWDGE rings (only sync + scalar have them on trn2); the
            # scalar engine just fires the triggers up-front, before any exp
            for dst, src in sync_q:
                nc.sync.dma_start(dst, src)
            for dst, src in scal_q:
                nc.scalar.dma_start(dst, src)

            def x_mv(nt, ko):
                """[128, 2, 256] moving view of x tokens [nt*512,(nt+1)*512)"""
                return x_sb[:, 2 * nt:2 * nt + 2, ko, :]

            # rope: dst = raw*cos + rotate_half(raw)*sin. The rotate-half is
            # a PE matmul against a constant 128x128 permutation matrix
            # (rot_ps = PRM.T @ raw, 213ns) — no DMA ring traffic, no scalar
            # engine. It reads the bf16 cast, so it is software-pipelined:
            # the perm matmul + combine of rope i are emitted at the start
            # of the NEXT unit (flush_rope), hiding the DVE-cast latency.
            rope_pending = []

            def flush_rope():
                while rope_pending:
                    rope_pending.pop(0)()

            def rope(dst, nt):
                ts = slice(nt * 512, (nt + 1) * 512)

                def fin(ps):
                    raw = work.tile([P, 512], BF16, tag="ropraw")
                    nc.vector.tensor_copy(raw[:], ps[:])

                    def finish():
                        rps = big_ps.tile([P, 512], F32, tag="big")
                        nc.tensor.matmul(rps[:], prm_sb[:], raw[:],
                                         start=True, stop=True)
                        t1 = work.tile([P, 512], BF16, tag="ropt1")
                        nc.vector.tensor_mul(t1[:], raw[:], cos_sb[:, ts])
                        rtb = work.tile([P, 512], BF16, tag="roprtb")
                        nc.vector.tensor_mul(rtb[:], rps[:], sin_sb[:, ts])
                        nc.vector.tensor_add(dst[:, ts], t1[:], rtb[:])
                    rope_pending.append(finish)
                return fin

            # ---- K projection + rope (feature-major K^T [128, T]) ----
            kt = const.tile([P, T], BF16, tag="kt")

            def k_proj(nt):
                flush_rope()
                ps = big_ps.tile([P, 512], F32, tag="big")
                for ko in range(KO):
                    nc.tensor.matmul(ps[:], wk_sb[:, ko, :], x_mv(nt, ko),
                                     start=(ko == 0), stop=(ko == KO - 1))
                rope(kt, nt)(ps)

            # ---- V projection (token-major, 64 ones columns per head) ----
            def v_proj(tt):
                flush_rope()
                ps = big_ps.tile([P, DKV], F32, tag="big")
                for ko in range(KO):
                    nc.tensor.matmul(
                        ps[:], x_sb[:, tt // 2, ko,
                                    (tt % 2) * P:(tt % 2) * P + P],
                        wv_sb[:, ko, :],
                        start=(ko == 0), stop=(ko == KO - 1))
                nc.vector.tensor_copy(v_sb[:, tt, 0:64], ps[:, 0:64])
                nc.vector.tensor_copy(v_sb[:, tt, 128:192], ps[:, 64:128])

            # ---- Q projection + rope for one head pair, one token tile ----
            qts = {}
            for j in range(4):
                qts[j] = const.tile([P, T], BF16, tag=f"qt{j}", name=f"qt{j}")

            def q_proj_nt(j, nt):
                flush_rope()
                ps = big_ps.tile([P, 512], F32, tag="big")
                for ko in range(KO):
                    nc.tensor.matmul(ps[:], wq_sb[:, j, ko, :], x_mv(nt, ko),
                                     start=(ko == 0), stop=(ko == KO - 1))
                rope(qts[j], nt)(ps)

            # ---- filler queue: PE work pulled between attention pairs.
            # Entries carry an estimated PE cost (us); pull(budget) drains
            # ~budget worth of work, carrying surplus credit so chunky units
            # (3.4us q-projections) average out over iterations ----
            pool = deque()
            pull_credit = [0.0]

            def pull(budget):
                pull_credit[0] += budget
                while pool and pull_credit[0] > 0:
                    cost, fn = pool.popleft()
                    fn()
                    pull_credit[0] -= cost

            def drain():
                while pool:
                    pool.popleft()[1]()
                pull_credit[0] = 0.0

            # ---- attention for one (qt, j) head-pair into ot tile ----
            def attn_block(qt, j, ot, split_epi=False):
                flush_rope()
                pv = pv_ps.tile([P, 1024], F32, tag="pv")
                nkb = 4 * qt + 4

                def flush_pv(prev):
                    # PV matmuls for the previous kb (software pipeline: issued
                    # after the next kb's scores so PE never waits on ACT's exp
                    # of the current block). Diagonal blocks only touch output
                    # columns >= their first causally-valid query. (Splitting
                    # each PV into two concurrent K=64 row-tiles crashes the
                    # device - two in-flight matmuls may not share a psum bank.)
                    pkb, c0, pp = prev
                    ppv = pp[:].rearrange("p (two t) -> p two t", two=2)
                    nc.tensor.matmul(pv[:, c0:512], v_sb[:, pkb, 0:128],
                                     ppv[:, 0, c0:512],
                                     start=(pkb == 0), stop=(pkb == nkb - 1))
                    nc.tensor.matmul(pv[:, 512 + c0:1024], v_sb[:, pkb, 128:256],
                                     ppv[:, 1, c0:512],
                                     start=(pkb == 0), stop=(pkb == nkb - 1))

                pending = []
                for kb0 in range(0, nkb, 2):
                    # issue TWO key-blocks' score pairs back-to-back: the
                    # later kt LDWEIGHTS pull ahead behind score matmuls in
                    # the other row group (a full-row PV matmul in between
                    # would block the pull-ahead and expose ~107ns each)
                    sps = []
                    for kb in (kb0, kb0 + 1):
                        tk = slice(kb * P, (kb + 1) * P)
                        jr = kb - 4 * qt       # >= 0 on diagonal blocks
                        c0 = max(0, jr) * P    # first causally-valid column
                        tqs = slice(qt * 512 + c0, (qt + 1) * 512)
                        # one 2-bank psum tile holds both heads' scores; the
                        # two matmuls write disjoint banks, then a SINGLE exp
                        # (3-dim AP) and a single broadcast mask cover both
                        # halves, halving the pacing-engine instruction count
                        sp = s_ps.tile([P, 1024], F32, tag="s")
                        spv = sp[:].rearrange("p (two t) -> p two t", two=2)
                        nc.tensor.matmul(sp[:, c0:512], kt[0:64, tk],
                                         qts[j][0:64, tqs],
                                         start=True, stop=True)
                        nc.tensor.matmul(sp[:, 512 + c0:1024], kt[64:128, tk],
                                         qts[j][64:128, tqs],
                                         start=True, stop=True)
                        sps.append((kb, c0, jr, spv))
                    while pending:
                        flush_pv(pending.pop(0))
                    for kb, c0, jr, spv in sps:
                        pp = pexp.tile([P, 1024], BF16, tag="p")
                        ppv = pp[:].rearrange("p (two t) -> p two t", two=2)
                        nc.scalar.activation(ppv[:, :, c0:512],
                                             spv[:, :, c0:512],
                                             AF.Exp, scale=SCALE)
                        if jr >= 0:
                            # triangle mask on the partially-valid block
                            nc.vector.tensor_mul(
                                ppv[:, :, c0:c0 + P], ppv[:, :, c0:c0 + P],
                                tri_sb[:, None, :].to_broadcast((P, 2, P)))
                        pending.append((kb, c0, pp))
                    pull(1.0)
                for pr in pending:
                    flush_pv(pr)
                # normalization: rows 64..127 of pv hold 64 copies of the
                # sum-exp row (ones trick): stage to SBUF, one wide
                # reciprocal, then fused multiplies finalize ot from PSUM
                den = rrp.tile([64, 1024], F32, tag="den")
                rec = rrp.tile([64, 1024], F32, tag="rec")
                if split_epi:
                    # per-head chain on the tail block: head0's normalize
                    # overlaps head1's final PV matmul
                    for hh in range(2):
                        cs = slice(hh * 512, hh * 512 + 512)
                        nc.vector.tensor_copy(den[:, cs], pv[64:128, cs])
                        nc.vector.reciprocal_approx_fast(rec[:, cs],
                                                         den[:, cs])
                        nc.vector.tensor_mul(ot[hh * 64:hh * 64 + 64, j, :],
                                             pv[0:64, cs], rec[:, cs])
                else:
                    nc.vector.tensor_copy(den[:], pv[64:128, :])
                    nc.vector.reciprocal_approx_fast(rec[:], den[:])
                    nc.vector.tensor_mul(ot[0:64, j, :], pv[0:64, 0:512],
                                         rec[:, 0:512])
                    nc.vector.tensor_mul(ot[64:128, j, :], pv[0:64, 512:1024],
                                         rec[:, 512:1024])

            # ---- Wo output projection, emitted as per-oc filler units ----
            ysbs = {}

            def wo_unit(qt, tt, oc, ot):
                flush_rope()
                r0 = qt * 512 + tt * P
                if oc == 0:
                    ysbs[(qt, tt)] = ysp.tile([P, D], BF16, tag="ysb",
                                              name=f"ysb{qt}_{tt}")
                ysb = ysbs[(qt, tt)]
                yps = big_ps.tile([P, 512], F32, tag="big")
                for kf in range(4):
                    nc.tensor.matmul(yps[:], ot[:, kf, tt * P:(tt + 1) * P],
                                     wo_sb[:, kf, oc * 512:(oc + 1) * 512],
                                     start=(kf == 0), stop=(kf == 3))
                nc.vector.tensor_copy(ysb[:, oc * 512:(oc + 1) * 512], yps[:])
                if qt == 2:
                    # tail blocks: ship each oc chunk immediately so the
                    # final drain is one 128KB chunk, not a 0.5MB row
                    nc.sync.dma_start(y[r0:r0 + P, oc * 512:(oc + 1) * 512],
                                      ysb[:, oc * 512:(oc + 1) * 512])
                elif oc == 3:
                    # one whole-row DMA (4KB descriptors) on the otherwise
                    # idle sync engine; scalar must stay free for exp
                    nc.sync.dma_start(y[r0:r0 + P, :], ysb[:])

            def queue_wo(qt):
                ot = ot_tiles[qt]
                for tt in range(4):
                    for oc in range(4):
                        pool.append((0.85,
                                     lambda qt=qt, tt=tt, oc=oc, ot=ot:
                                     wo_unit(qt, tt, oc, ot)))

            # ---- emission: dependency wavefront ----
            # wave 0 (x slices 0,1 + wk/wv/wq/cos/sin): V, K, all-j Q for nt0
            v_proj(0); v_proj(1); v_proj(2); v_proj(3)
            k_proj(0)
            for j in range(4):
                q_proj_nt(j, 0)

            ot_tiles = {qt: otp.tile([P, 4, 512], BF16, tag="ot",
                                     name=f"ot{qt}") for qt in range(4)}

            # region 1: attn qt0; filler = wave 1 (x slices 2,3)
            for f in [(0.9, lambda: v_proj(4)), (0.9, lambda: v_proj(5)),
                      (0.9, lambda: v_proj(6)), (0.9, lambda: v_proj(7)),
                      (3.4, lambda: k_proj(1)),
                      (3.4, lambda: q_proj_nt(0, 1)),
                      (3.4, lambda: q_proj_nt(1, 1)),
                      (3.4, lambda: q_proj_nt(2, 1)),
                      (3.4, lambda: q_proj_nt(3, 1))]:
                pool.append(f)
            for j in range(4):
                attn_block(0, j, ot_tiles[0])
                pull(1.0)
            drain()             # qt1 needs all of wave 1

            # region 2: attn qt1; filler = wave 3 (x slices 6,7), which must
            # fully precede qt3's attention, so it drains here
            for f in [(0.9, lambda: v_proj(12)), (0.9, lambda: v_proj(13)),
                      (0.9, lambda: v_proj(14)), (0.9, lambda: v_proj(15)),
                      (3.4, lambda: k_proj(3)),
                      (3.4, lambda: q_proj_nt(0, 3)),
                      (3.4, lambda: q_proj_nt(1, 3)),
                      (3.4, lambda: q_proj_nt(2, 3)),
                      (3.4, lambda: q_proj_nt(3, 3))]:
                pool.append(f)
            for j in range(4):
                attn_block(1, j, ot_tiles[1])
                pull(1.0)
            drain()

            # region 3: attn qt3 (heaviest exp load). Filler = wave 2 (only
            # needed by qt2, i.e. region 4) + Wo(qt0): enough PE work that
            # the scores never stall on the exp double-buffer.
            for f in [(0.9, lambda: v_proj(8)), (0.9, lambda: v_proj(9)),
                      (0.9, lambda: v_proj(10)), (0.9, lambda: v_proj(11)),
                      (3.4, lambda: k_proj(2)),
                      (3.4, lambda: q_proj_nt(0, 2)),
                      (3.4, lambda: q_proj_nt(1, 2)),
                      (3.4, lambda: q_proj_nt(2, 2)),
                      (3.4, lambda: q_proj_nt(3, 2))]:
                pool.append(f)
            queue_wo(0)
            for j in range(4):
                attn_block(3, j, ot_tiles[3])
                pull(1.0)
            drain()

            # region 4: attn qt2; filler = Wo(qt1) + Wo(qt3)
            queue_wo(1)
            queue_wo(3)
            for j in range(4):
                attn_block(2, j, ot_tiles[2],
                           split_epi=(j == 3))
                pull(1.0)
            drain()

            # tail: Wo(qt2)
            queue_wo(2)
            drain()

    nc.finalize()
    _nc_cache["nc"] = nc
    return nc


def make_in_maps(x, Wq, Wk, Wv, Wo):
    bf = ml_dtypes.bfloat16
    x = np.asarray(x, np.float32)
    Wq = np.asarray(Wq, np.float32)
    Wk = np.asarray(Wk, np.float32)
    Wv = np.asarray(Wv, np.float32)
    Wo = np.asarray(Wo, np.float32)

    # rope tables, [128, T]: row p covers head-dim d = p % 64
    half = HD // 2
    inv_freq = 1.0 / (ROPE_BASE ** (np.arange(half, dtype=np.float64) / half))
    pos = np.arange(T, dtype=np.float64)
    d_idx = np.arange(P) % HD
    freqs = pos[None, :] * inv_freq[d_idx % half][:, None]      # [128, T]
    cos_t = np.cos(freqs).astype(bf)
    sign = np.where(d_idx < half, -1.0, 1.0)[:, None]
    sin_t = (np.sin(freqs) * sign).astype(bf)

    # causal 0/1 triangle for the partially-valid diagonal sub-block
    pp = np.arange(P)[:, None]
    ff = np.arange(P)[None, :]
    tri = (ff >= pp).astype(bf)

    # rotate-half permutation matrix: prm[k, m] = 1 iff k == rot(m), so the
    # PE matmul prm.T @ raw yields raw[rot(m)] on partition m (sign lives in
    # the sin table)
    m_idx = np.arange(P)
    rot_m = np.where(m_idx % HD < half, m_idx + half, m_idx - half)
    prm = np.zeros((P, P), np.float32)
    prm[rot_m, m_idx] = 1.0
    prm = prm.astype(bf)

    def pack(a, n_chunks):
        # [n_chunks*128, F] -> [128, n_chunks*F] partition-major
        F = a.shape[1]
        return np.ascontiguousarray(
            a.reshape(n_chunks, P, F).transpose(1, 0, 2).reshape(P, n_chunks * F))

    in_maps = []
    for c in range(8):
        b, g = c // 4, c % 4
        heads = [8 * g + h for h in PERM_Q]
        qrows = np.concatenate([np.arange(h * HD, (h + 1) * HD) for h in heads])
        kvrows = np.arange(2 * g * HD, (2 * g + 2) * HD)
        xT = np.ascontiguousarray(x[b].T).astype(bf)             # [D, T]
        wqT = np.ascontiguousarray(Wq[qrows, :].T).astype(bf)    # [D, FQ]
        wkT = np.ascontiguousarray(Wk[kvrows, :].T).astype(bf)
        wvT = np.ascontiguousarray(Wv[kvrows, :].T).astype(bf)
        woT = np.ascontiguousarray(Wo[:, qrows].T).astype(bf)    # [FQ, D]
        # xS rows = e*128 + p, cols = ko*TE + t  (slice e, token e*TE+t)
        xs = np.ascontiguousarray(
            xT.reshape(KO, P, NXE, TE).transpose(2, 1, 0, 3)
            .reshape(NXE * P, KO * TE))
        # wqS rows = j*128 + p, cols = ko*128 + f
        wqs = np.ascontiguousarray(
            wqT.reshape(KO, P, 4, P).transpose(2, 1, 0, 3).reshape(4 * P, KO * P))
        in_maps.append({
            "xS": xs,
            "wqS": wqs,
            "wkS": pack(wkT, KO),
            "wvS": pack(wvT, KO),
            "woS": pack(woT, 4),
            "cosT": cos_t,
            "sinT": sin_t,
            "tri": tri,
            "prm": prm,
        })
    return in_maps


def combine_outputs(results):
    out = np.zeros((B, T, D), np.float32)
    for c in range(8):
        out[c // 4] += np.asarray(results[c]["y"], np.float32)
    return out


def _ensure_ntff_hook():
    """Register the axon NTFF profile hook (antenv.axon_hooks is missing
    from this image; recreate it and wire the ctypes hook from trn_boot)."""
    import sys, types
    if "antenv.axon_hooks" in sys.modules:
        return
    m = types.ModuleType("antenv.axon_hooks")
    hook = [None]
    m.set_axon_ntff_profile_hook = lambda h: hook.__setitem__(0, h)
    m.get_axon_ntff_profile_hook = lambda: hook[0]
    sys.modules["antenv.axon_hooks"] = m
    import antenv
    antenv.axon_hooks = m
    sys.path.insert(0, "/root/.axon_site")
    from trn_agent_boot.trn_boot import _ntff_profile_via_ctypes
    m.set_axon_ntff_profile_hook(
        _ntff_profile_via_ctypes("/opt/axon/libaxon_pjrt.so"))


def kernel(x, Wq, Wk, Wv, Wo, _trace=False):
    if _trace:
        _ensure_ntff_hook()
    nc = build_nc()
    in_maps = make_in_maps(x, Wq, Wk, Wv, Wo)
    res = run_bass_kernel_spmd(nc, in_maps, core_ids=list(range(8)), trace=_trace)
    out = combine_outputs(res.results)
    if _trace:
        return out, res
    return out


# revision 37
# speedup vs baseline: 1.2111x; 1.0035x over previous
"""Distributed GQA attention kernel for Trainium2 (8 NeuronCores).

Sharding: 2-way data parallel over batch x 4-way tensor parallel over heads.
Core c handles batch b = c // 4 and head group g = c % 4 (8 q-heads, 2 kv-heads).
Each core computes a full-size partial of the output (its head group pushed
through Wo); the host sums the 4 partials per batch. No on-device collective.

Device-side layout is feature-major (Q^T/K^T: [feature partitions, T free]) so
projections consume the host-pre-transposed x^T directly, attention scores are
computed transposed (S^T[tk, tq]) so softmax(P)@V needs no transposes, and the
softmax denominator is broadcast for free by 64 ones-columns appended to V
(the PV matmul then emits 64 identical sum-exp rows; normalization is a copy +
one [64,1024] reciprocal + fused multiplies out of PSUM; reciprocal must read
SBUF - the custom-DVE op returns garbage on a partition-offset PSUM source).
Score matmuls are K=64 so the two heads' matmuls land on different PE row
groups (base partitions 0/64) and stream CONCURRENTLY (2x packing).

Schedule: the kernel is PE-streaming-bound (~224us of matmul columns at
2.4GHz) with the scalar engine (exp: (N+352)/1.2ns per instruction, ~163us
total) second. The emission is a dependency wavefront: x arrives on the two
HWDGE rings (sync+scalar — the only engines with hardware DGE); ~80 dummy
matmuls warm the PE HAM clock-gate (cold=1.2GHz) while the first slices land;
V/K/Q projections chase the slices; attention for query-tile qt starts as
soon as its K/Q columns exist. Attention inner loops pull cost-metered
"filler" PE work (remaining projections, Wo output blocks) from a queue
between score/PV pairs so the PE never idles while ACT chews exp: the ACT
deficit is ~1us per 2-key-block iteration. ACT runs nothing but exp — rope's
psum cast runs on DVE and rotate-half is a PE matmul against a constant
128x128 permutation (software-pipelined behind the cast via flush_rope, so
the PE never waits on DVE). qt region order 0,1,3,2 balances each region's
exp load against available filler (wave3 projections must drain during qt1
since qt3 needs them; wave2 defers into the qt3 region as filler); Wo(qt2)
plus its output DMA is the only tail. Output y rides the idle sync engine
only — a dma_start occupies its issuing engine ~5ns/descriptor, which would
starve exp if placed on scalar.

Note: the chip randomly enters a ~2.0GHz power state (vs 2.4) for whole runs;
measured times swing ~15% run-to-run on identical code.
"""

import numpy as np
import ml_dtypes
from collections import deque
from contextlib import ExitStack

import concourse.bass as bass
from concourse import bacc
import concourse.mybir as mybir
import concourse.tile as tile
from concourse.bass_utils import run_bass_kernel_spmd

BF16 = mybir.dt.bfloat16
F32 = mybir.dt.float32
AF = mybir.ActivationFunctionType

P = 128
B, T, D = 2, 2048, 2048
NUM_HEADS, NUM_KV_HEADS, HD = 32, 8, 64
FQ = 512          # q features per core (8 heads x 64)
DKV = 128         # kv features per core (2 kv heads x 64)
KO = D // P       # 16 contraction tiles over d_model
NT = T // 512     # 4 tiles of 512 along T
NXE = 8           # x arrives in 8 T-slices of 256
TE = T // NXE
SCALE = 1.0 / np.sqrt(HD)
ROPE_BASE = 10000.0
# local head order inside the 512 q-features: pairs (j, j+4) so that the two
# heads in partition tile j sit at bases 0/64 matching kv heads 0/1 in K^T
PERM_Q = [0, 4, 1, 5, 2, 6, 3, 7]

_nc_cache = {}


def build_nc():
    if "nc" in _nc_cache:
        return _nc_cache["nc"]
    nc = bacc.Bacc()
    # host-packed layouts: row = slice*128 + partition, all loads contiguous
    xS = nc.declare_dram_parameter("xS", [NXE * P, KO * TE], BF16, isOutput=False)
    wqS = nc.declare_dram_parameter("wqS", [4 * P, KO * P], BF16, isOutput=False)
    wkS = nc.declare_dram_parameter("wkS", [P, KO * DKV], BF16, isOutput=False)
    wvS = nc.declare_dram_parameter("wvS", [P, KO * DKV], BF16, isOutput=False)
    woS = nc.declare_dram_parameter("woS", [P, 4 * D], BF16, isOutput=False)
    cosd = nc.declare_dram_parameter("cosT", [P, T], BF16, isOutput=False)
    sind = nc.declare_dram_parameter("sinT", [P, T], BF16, isOutput=False)
    mskd = nc.declare_dram_parameter("tri", [P, P], BF16, isOutput=False)
    prmd = nc.declare_dram_parameter("prm", [P, P], BF16, isOutput=False)
    y = nc.declare_dram_parameter("y", [T, D], BF16, isOutput=True)

    with tile.TileContext(nc) as tc:
        with ExitStack() as ctx:
            const = ctx.enter_context(tc.tile_pool(name="const", bufs=1))
            work = ctx.enter_context(tc.tile_pool(name="work", bufs=6))
            otp = ctx.enter_context(tc.tile_pool(name="otp", bufs=2))
            pexp = ctx.enter_context(tc.tile_pool(name="pexp", bufs=8))
            rrp = ctx.enter_context(tc.tile_pool(name="rrp", bufs=2))
            ysp = ctx.enter_context(tc.tile_pool(name="ysp", bufs=2))
            big_ps = ctx.enter_context(tc.tile_pool(name="bigps", bufs=2, space="PSUM"))
            pv_ps = ctx.enter_context(tc.tile_pool(name="pvps", bufs=1, space="PSUM"))
            s_ps = ctx.enter_context(tc.tile_pool(name="sps", bufs=2, space="PSUM"))

            x_sb = const.tile([P, NXE, KO, TE], BF16, tag="x")
            wq_sb = const.tile([P, 4, KO, P], BF16, tag="wq")
            wv_sb = const.tile([P, KO, DKV], BF16, tag="wv")
            wk_sb = const.tile([P, KO, DKV], BF16, tag="wk")
            cos_sb = const.tile([P, T], BF16, tag="cos")
            sin_sb = const.tile([P, T], BF16, tag="sin")
            tri_sb = const.tile([P, P], BF16, tag="tri")
            prm_sb = const.tile([P, P], BF16, tag="prm")
            wo_sb = const.tile([P, 4, D], BF16, tag="wo")
            warm_sb = const.tile([P, P], BF16, tag="warm")

            # ---- V layout + PE warm-up (emitted first: gpsimd memsets, then
            # dummy matmuls keep the PE busy from ~6.4us so the HAM clock-gate
            # reaches 8/8 before real data lands, and stays there) ----
            v_sb = const.tile([P, 16, 256], BF16, tag="v")
            nc.gpsimd.memset(warm_sb[:], 0.0)
            nc.gpsimd.memset(v_sb[:, :, 64:128], 1.0)
            nc.gpsimd.memset(v_sb[:, :, 192:256], 1.0)
            wps = big_ps.tile([P, P], F32, tag="big")
            for _ in range(82):
                nc.tensor.matmul(wps[:], warm_sb[:], warm_sb[:],
                                 start=True, stop=True)

            # ---- input loads: two HWDGE rings (sync + gpsimd), x slices
            # split into partition halves so both rings carry every slice;
            # weights slotted just before first need ----
            def xs_half(e, h):
                return xS[e * P + 64 * h:e * P + 64 * (h + 1), :].rearrange(
                    "p (k t) -> p k t", k=KO)

            def wq_load(j):
                return wqS[j * P:(j + 1) * P, :].rearrange(
                    "p (k f) -> p k f", k=KO)

            def xs_full(e):
                return xS[e * P:(e + 1) * P, :].rearrange(
                    "p (k t) -> p k t", k=KO)

            # slices 0,1 split across both rings (halves the wave-0 latency);
            # later slices whole, alternating; weights slotted before first
            # need per the measured ~0.17 MB/us per-ring arrival rate
            def xs_q(e, q):
                return xS[e * P + 32 * q:e * P + 32 * (q + 1), :].rearrange(
                    "p (k t) -> p k t", k=KO)

            # slice 0 split into quarters across both rings so the first V
            # matmul can start ~14.4us instead of ~19; V/K-critical weights
            # ride the sync ring (it starts ~2us earlier)
            wv_src = wvS[:].rearrange("p (k f) -> p k f", k=KO)
            sync_q = [
                (x_sb[0:32, 0], xs_q(0, 0)),
                (x_sb[32:64, 0], xs_q(0, 1)),
                (wv_sb[0:64], wv_src[0:64]),
                (x_sb[0:64, 1], xs_half(1, 0)),
                (wk_sb[:], wkS[:].rearrange("p (k f) -> p k f", k=KO)),
                (wq_sb[:, 0], wq_load(0)),
                (wq_sb[:, 2], wq_load(2)),
                (x_sb[:, 2], xs_full(2)),
                (x_sb[:, 4], xs_full(4)),
                (x_sb[:, 6], xs_full(6)),
            ]
            scal_q = [
                (x_sb[64:96, 0], xs_q(0, 2)),
                (x_sb[96:128, 0], xs_q(0, 3)),
                (wv_sb[64:128], wv_src[64:128]),
                (x_sb[64:128, 1], xs_half(1, 1)),
                (wq_sb[:, 1], wq_load(1)),
                (prm_sb[:], prmd[:]),
                (tri_sb[:], mskd[:]),
                (cos_sb[:], cosd[:]),
                (sin_sb[:], sind[:]),
                (wq_sb[:, 3], wq_load(3)),
                (x_sb[:, 3], xs_full(3)),
                (x_sb[:, 5], xs_full(5)),
                (x_sb[:, 7], xs_full(7)),
                (wo_sb[:], woS[:].rearrange("p (k d) -> p k d", k=4)),
            ]
            # both HWDGE rings (only sync + scalar have them on trn2); the
            # scalar engine just fires the triggers up-front, before any exp
            for dst, src in sync_q:
                nc.sync.dma_start(dst, src)
            for dst, src in scal_q:
                nc.scalar.dma_start(dst, src)

            def x_mv(nt, ko):
                """[128, 2, 256] moving view of x tokens [nt*512,(nt+1)*512)"""
                return x_sb[:, 2 * nt:2 * nt + 2, ko, :]

            # rope: dst = raw*cos + rotate_half(raw)*sin. The rotate-half is
            # a PE matmul against a constant 128x128 permutation matrix
            # (rot_ps = PRM.T @ raw, 213ns) — no DMA ring traffic, no scalar
            # engine. It reads the bf16 cast, so it is software-pipelined:
            # the perm matmul + combine of rope i are emitted at the start
            # of the NEXT unit (flush_rope), hiding the DVE-cast latency.
            rope_pending = []

            def flush_rope():
                while rope_pending:
                    rope_pending.pop(0)()

            def rope(dst, nt):
                ts = slice(nt * 512, (nt + 1) * 512)

                def fin(ps):
                    raw = work.tile([P, 512], BF16, tag="ropraw")
                    nc.vector.tensor_copy(raw[:], ps[:])

                    def finish():
                        rps = big_ps.tile([P, 512], F32, tag="big")
                        nc.tensor.matmul(rps[:], prm_sb[:], raw[:],
                                         start=True, stop=True)
                        t1 = work.tile([P, 512], BF16, tag="ropt1")
                        nc.vector.tensor_mul(t1[:], raw[:], cos_sb[:, ts])
                        rtb = work.tile([P, 512], BF16, tag="roprtb")
                        nc.vector.tensor_mul(rtb[:], rps[:], sin_sb[:, ts])
                        nc.vector.tensor_add(dst[:, ts], t1[:], rtb[:])
                    rope_pending.append(finish)
                return fin

            # ---- K projection + rope (feature-major K^T [128, T]) ----
            kt = const.tile([P, T], BF16, tag="kt")

            def k_proj(nt):
                flush_rope()
                ps = big_ps.tile([P, 512], F32, tag="big")
                for ko in range(KO):
                    nc.tensor.matmul(ps[:], wk_sb[:, ko, :], x_mv(nt, ko),
                                     start=(ko == 0), stop=(ko == KO - 1))
                rope(kt, nt)(ps)

            # ---- V projection (token-major, 64 ones columns per head) ----
            def v_proj(tt):
                flush_rope()
                ps = big_ps.tile([P, DKV], F32, tag="big")
                for ko in range(KO):
                    nc.tensor.matmul(
                        ps[:], x_sb[:, tt // 2, ko,
                                    (tt % 2) * P:(tt % 2) * P + P],
                        wv_sb[:, ko, :],
                        start=(ko == 0), stop=(ko == KO - 1))
                nc.vector.tensor_copy(v_sb[:, tt, 0:64], ps[:, 0:64])
                nc.vector.tensor_copy(v_sb[:, tt, 128:192], ps[:, 64:128])

            # ---- Q projection + rope for one head pair, one token tile ----
            qts = {}
            for j in range(4):
                qts[j] = const.tile([P, T], BF16, tag=f"qt{j}", name=f"qt{j}")

            def q_proj_nt(j, nt):
                flush_rope()
                ps = big_ps.tile([P, 512], F32, tag="big")
                for ko in range(KO):
                    nc.tensor.matmul(ps[:], wq_sb[:, j, ko, :], x_mv(nt, ko),
                                     start=(ko == 0), stop=(ko == KO - 1))
                rope(qts[j], nt)(ps)

            # ---- filler queue: PE work pulled between attention pairs.
            # Entries carry an estimated PE cost (us); pull(budget) drains
            # ~budget worth of work, carrying surplus credit so chunky units
            # (3.4us q-projections) average out over iterations ----
            pool = deque()
            pull_credit = [0.0]

            def pull(budget):
                pull_credit[0] += budget
                while pool and pull_credit[0] > 0:
                    cost, fn = pool.popleft()
                    fn()
                    pull_credit[0] -= cost

            def drain():
                while pool:
                    pool.popleft()[1]()
                pull_credit[0] = 0.0

            # ---- attention for one (qt, j) head-pair into ot tile ----
            def attn_block(qt, j, ot, split_epi=False, pull_budget=1.0):
                flush_rope()
                pv = pv_ps.tile([P, 1024], F32, tag="pv")
                nkb = 4 * qt + 4

                def flush_pv(prev):
                    # PV matmuls for the previous kb (software pipeline: issued
                    # after the next kb's scores so PE never waits on ACT's exp
                    # of the current block). Diagonal blocks only touch output
                    # columns >= their first causally-valid query. (Splitting
                    # each PV into two concurrent K=64 row-tiles crashes the
                    # device - two in-flight matmuls may not share a psum bank.)
                    pkb, c0, pp = prev
                    ppv = pp[:].rearrange("p (two t) -> p two t", two=2)
                    nc.tensor.matmul(pv[:, c0:512], v_sb[:, pkb, 0:128],
                                     ppv[:, 0, c0:512],
                                     start=(pkb == 0), stop=(pkb == nkb - 1))
                    nc.tensor.matmul(pv[:, 512 + c0:1024], v_sb[:, pkb, 128:256],
                                     ppv[:, 1, c0:512],
                                     start=(pkb == 0), stop=(pkb == nkb - 1))

                pending = []
                for kb0 in range(0, nkb, 2):
                    # issue TWO key-blocks' score pairs back-to-back: the
                    # later kt LDWEIGHTS pull ahead behind score matmuls in
                    # the other row group (a full-row PV matmul in between
                    # would block the pull-ahead and expose ~107ns each)
                    sps = []
                    for kb in (kb0, kb0 + 1):
                        tk = slice(kb * P, (kb + 1) * P)
                        jr = kb - 4 * qt       # >= 0 on diagonal blocks
                        c0 = max(0, jr) * P    # first causally-valid column
                        tqs = slice(qt * 512 + c0, (qt + 1) * 512)
                        # one 2-bank psum tile holds both heads' scores; the
                        # two matmuls write disjoint banks, then a SINGLE exp
                        # (3-dim AP) and a single broadcast mask cover both
                        # halves, halving the pacing-engine instruction count
                        sp = s_ps.tile([P, 1024], F32, tag="s")
                        spv = sp[:].rearrange("p (two t) -> p two t", two=2)
                        nc.tensor.matmul(sp[:, c0:512], kt[0:64, tk],
                                         qts[j][0:64, tqs],
                                         start=True, stop=True)
                        nc.tensor.matmul(sp[:, 512 + c0:1024], kt[64:128, tk],
                                         qts[j][64:128, tqs],
                                         start=True, stop=True)
                        sps.append((kb, c0, jr, spv))
                    while pending:
                        flush_pv(pending.pop(0))
                    for kb, c0, jr, spv in sps:
                        pp = pexp.tile([P, 1024], BF16, tag="p")
                        ppv = pp[:].rearrange("p (two t) -> p two t", two=2)
                        nc.scalar.activation(ppv[:, :, c0:512],
                                             spv[:, :, c0:512],
                                             AF.Exp, scale=SCALE)
                        if jr >= 0:
                            # triangle mask on the partially-valid block
                            nc.vector.tensor_mul(
                                ppv[:, :, c0:c0 + P], ppv[:, :, c0:c0 + P],
                                tri_sb[:, None, :].to_broadcast((P, 2, P)))
                        pending.append((kb, c0, pp))
                    pull(pull_budget)
                for pr in pending:
                    flush_pv(pr)
                # normalization: rows 64..127 of pv hold 64 copies of the
                # sum-exp row (ones trick): stage to SBUF, one wide
                # reciprocal, then fused multiplies finalize ot from PSUM
                den = rrp.tile([64, 1024], F32, tag="den")
                rec = rrp.tile([64, 1024], F32, tag="rec")
                if split_epi:
                    # per-head chain on the tail block: head0's normalize
                    # overlaps head1's final PV matmul
                    for hh in range(2):
                        cs = slice(hh * 512, hh * 512 + 512)
                        nc.vector.tensor_copy(den[:, cs], pv[64:128, cs])
                        nc.vector.reciprocal_approx_fast(rec[:, cs],
                                                         den[:, cs])
                        nc.vector.tensor_mul(ot[hh * 64:hh * 64 + 64, j, :],
                                             pv[0:64, cs], rec[:, cs])
                else:
                    nc.vector.tensor_copy(den[:], pv[64:128, :])
                    nc.vector.reciprocal_approx_fast(rec[:], den[:])
                    nc.vector.tensor_mul(ot[0:64, j, :], pv[0:64, 0:512],
                                         rec[:, 0:512])
                    nc.vector.tensor_mul(ot[64:128, j, :], pv[0:64, 512:1024],
                                         rec[:, 512:1024])

            # ---- Wo output projection, emitted as per-oc filler units ----
            ysbs = {}

            def wo_unit(qt, tt, oc, ot):
                flush_rope()
                r0 = qt * 512 + tt * P
                if oc == 0:
                    ysbs[(qt, tt)] = ysp.tile([P, D], BF16, tag="ysb",
                                              name=f"ysb{qt}_{tt}")
                ysb = ysbs[(qt, tt)]
                yps = big_ps.tile([P, 512], F32, tag="big")
                for kf in range(4):
                    nc.tensor.matmul(yps[:], ot[:, kf, tt * P:(tt + 1) * P],
                                     wo_sb[:, kf, oc * 512:(oc + 1) * 512],
                                     start=(kf == 0), stop=(kf == 3))
                nc.vector.tensor_copy(ysb[:, oc * 512:(oc + 1) * 512], yps[:])
                if qt == 2:
                    # tail blocks: ship each oc chunk immediately so the
                    # final drain is one 128KB chunk, not a 0.5MB row
                    nc.sync.dma_start(y[r0:r0 + P, oc * 512:(oc + 1) * 512],
                                      ysb[:, oc * 512:(oc + 1) * 512])
                elif oc == 3:
                    # one whole-row DMA (4KB descriptors) on the otherwise
                    # idle sync engine; scalar must stay free for exp
                    nc.sync.dma_start(y[r0:r0 + P, :], ysb[:])

            def queue_wo(qt):
                ot = ot_tiles[qt]
                for tt in range(4):
                    for oc in range(4):
                        pool.append((0.85,
                                     lambda qt=qt, tt=tt, oc=oc, ot=ot:
                                     wo_unit(qt, tt, oc, ot)))

            # ---- emission: dependency wavefront ----
            # wave 0 (x slices 0,1 + wk/wv/wq/cos/sin): V, K, all-j Q for nt0
            v_proj(0); v_proj(1); v_proj(2); v_proj(3)
            k_proj(0)
            for j in range(4):
                q_proj_nt(j, 0)

            ot_tiles = {qt: otp.tile([P, 4, 512], BF16, tag="ot",
                                     name=f"ot{qt}") for qt in range(4)}

            # region 1: attn qt0; filler = wave 1 (x slices 2,3)
            for f in [(0.9, lambda: v_proj(4)), (0.9, lambda: v_proj(5)),
                      (0.9, lambda: v_proj(6)), (0.9, lambda: v_proj(7)),
                      (3.4, lambda: k_proj(1)),
                      (3.4, lambda: q_proj_nt(0, 1)),
                      (3.4, lambda: q_proj_nt(1, 1)),
                      (3.4, lambda: q_proj_nt(2, 1)),
                      (3.4, lambda: q_proj_nt(3, 1))]:
                pool.append(f)
            for j in range(4):
                attn_block(0, j, ot_tiles[0])
                pull(1.0)
            drain()             # qt1 needs all of wave 1

            # region 2: attn qt1; filler = wave 3 (x slices 6,7), which must
            # fully precede qt3's attention, so it drains here
            for f in [(0.9, lambda: v_proj(12)), (0.9, lambda: v_proj(13)),
                      (0.9, lambda: v_proj(14)), (0.9, lambda: v_proj(15)),
                      (3.4, lambda: k_proj(3)),
                      (3.4, lambda: q_proj_nt(0, 3)),
                      (3.4, lambda: q_proj_nt(1, 3)),
                      (3.4, lambda: q_proj_nt(2, 3)),
                      (3.4, lambda: q_proj_nt(3, 3))]:
                pool.append(f)
            for j in range(4):
                attn_block(1, j, ot_tiles[1])
                pull(1.0)
            drain()

            # region 3: attn qt3 (heaviest exp load). Filler = wave 2 (only
            # needed by qt2, i.e. region 4) + Wo(qt0): enough PE work that
            # the scores never stall on the exp double-buffer.
            for f in [(0.9, lambda: v_proj(8)), (0.9, lambda: v_proj(9)),
                      (0.9, lambda: v_proj(10)), (0.9, lambda: v_proj(11)),
                      (3.4, lambda: k_proj(2)),
                      (3.4, lambda: q_proj_nt(0, 2)),
                      (3.4, lambda: q_proj_nt(1, 2)),
                      (3.4, lambda: q_proj_nt(2, 2)),
                      (3.4, lambda: q_proj_nt(3, 2))]:
                pool.append(f)
            queue_wo(0)
            for j in range(4):
                attn_block(3, j, ot_tiles[3])
                pull(1.0)
            drain()

            # region 4: attn qt2; filler = Wo(qt1) + Wo(qt3)
            queue_wo(1)
            queue_wo(3)
            for j in range(4):
                attn_block(2, j, ot_tiles[2],
                           split_epi=(j == 3), pull_budget=0.8)
                pull(0.8)
            # leftovers (a few Wo(qt1/qt3) units) run HERE, covering the PE
            # while attn(2,3)'s normalize chain drains on the vector engine —
            # otherwise the first Wo(qt2) LDW stalls ~4.4us on it
            drain()

            # tail: Wo(qt2)
            queue_wo(2)
            drain()

    nc.finalize()
    _nc_cache["nc"] = nc
    return nc


def make_in_maps(x, Wq, Wk, Wv, Wo):
    bf = ml_dtypes.bfloat16
    x = np.asarray(x, np.float32)
    Wq = np.asarray(Wq, np.float32)
    Wk = np.asarray(Wk, np.float32)
    Wv = np.asarray(Wv, np.float32)
    Wo = np.asarray(Wo, np.float32)

    # rope tables, [128, T]: row p covers head-dim d = p % 64
    half = HD // 2
    inv_freq = 1.0 / (ROPE_BASE ** (np.arange(half, dtype=np.float64) / half))
    pos = np.arange(T, dtype=np.float64)
    d_idx = np.arange(P) % HD
    freqs = pos[None, :] * inv_freq[d_idx % half][:, None]      # [128, T]
    cos_t = np.cos(freqs).astype(bf)
    sign = np.where(d_idx < half, -1.0, 1.0)[:, None]
    sin_t = (np.sin(freqs) * sign).astype(bf)

    # causal 0/1 triangle for the partially-valid diagonal sub-block
    pp = np.arange(P)[:, None]
    ff = np.arange(P)[None, :]
    tri = (ff >= pp).astype(bf)

    # rotate-half permutation matrix: prm[k, m] = 1 iff k == rot(m), so the
    # PE matmul prm.T @ raw yields raw[rot(m)] on partition m (sign lives in
    # the sin table)
    m_idx = np.arange(P)
    rot_m = np.where(m_idx % HD < half, m_idx + half, m_idx - half)
    prm = np.zeros((P, P), np.float32)
    prm[rot_m, m_idx] = 1.0
    prm = prm.astype(bf)

    def pack(a, n_chunks):
        # [n_chunks*128, F] -> [128, n_chunks*F] partition-major
        F = a.shape[1]
        return np.ascontiguousarray(
            a.reshape(n_chunks, P, F).transpose(1, 0, 2).reshape(P, n_chunks * F))

    in_maps = []
    for c in range(8):
        b, g = c // 4, c % 4
        heads = [8 * g + h for h in PERM_Q]
        qrows = np.concatenate([np.arange(h * HD, (h + 1) * HD) for h in heads])
        kvrows = np.arange(2 * g * HD, (2 * g + 2) * HD)
        xT = np.ascontiguousarray(x[b].T).astype(bf)             # [D, T]
        wqT = np.ascontiguousarray(Wq[qrows, :].T).astype(bf)    # [D, FQ]
        wkT = np.ascontiguousarray(Wk[kvrows, :].T).astype(bf)
        wvT = np.ascontiguousarray(Wv[kvrows, :].T).astype(bf)
        woT = np.ascontiguousarray(Wo[:, qrows].T).astype(bf)    # [FQ, D]
        # xS rows = e*128 + p, cols = ko*TE + t  (slice e, token e*TE+t)
        xs = np.ascontiguousarray(
            xT.reshape(KO, P, NXE, TE).transpose(2, 1, 0, 3)
            .reshape(NXE * P, KO * TE))
        # wqS rows = j*128 + p, cols = ko*128 + f
        wqs = np.ascontiguousarray(
            wqT.reshape(KO, P, 4, P).transpose(2, 1, 0, 3).reshape(4 * P, KO * P))
        in_maps.append({
            "xS": xs,
            "wqS": wqs,
            "wkS": pack(wkT, KO),
            "wvS": pack(wvT, KO),
            "woS": pack(woT, 4),
            "cosT": cos_t,
            "sinT": sin_t,
            "tri": tri,
            "prm": prm,
        })
    return in_maps


def combine_outputs(results):
    out = np.zeros((B, T, D), np.float32)
    for c in range(8):
        out[c // 4] += np.asarray(results[c]["y"], np.float32)
    return out


def _ensure_ntff_hook():
    """Register the axon NTFF profile hook (antenv.axon_hooks is missing
    from this image; recreate it and wire the ctypes hook from trn_boot)."""
    import sys, types
    if "antenv.axon_hooks" in sys.modules:
        return
    m = types.ModuleType("antenv.axon_hooks")
    hook = [None]
    m.set_axon_ntff_profile_hook = lambda h: hook.__setitem__(0, h)
    m.get_axon_ntff_profile_hook = lambda: hook[0]
    sys.modules["antenv.axon_hooks"] = m
    import antenv
    antenv.axon_hooks = m
    sys.path.insert(0, "/root/.axon_site")
    from trn_agent_boot.trn_boot import _ntff_profile_via_ctypes
    m.set_axon_ntff_profile_hook(
        _ntff_profile_via_ctypes("/opt/axon/libaxon_pjrt.so"))


def kernel(x, Wq, Wk, Wv, Wo, _trace=False):
    if _trace:
        _ensure_ntff_hook()
    nc = build_nc()
    in_maps = make_in_maps(x, Wq, Wk, Wv, Wo)
    res = run_bass_kernel_spmd(nc, in_maps, core_ids=list(range(8)), trace=_trace)
    out = combine_outputs(res.results)
    if _trace:
        return out, res
    return out


# revision 38
# speedup vs baseline: 1.2119x; 1.0006x over previous
"""Distributed GQA attention kernel for Trainium2 (8 NeuronCores).

Sharding: 2-way data parallel over batch x 4-way tensor parallel over heads.
Core c handles batch b = c // 4 and head group g = c % 4 (8 q-heads, 2 kv-heads).
Each core computes a full-size partial of the output (its head group pushed
through Wo); the host sums the 4 partials per batch. No on-device collective.

Device-side layout is feature-major (Q^T/K^T: [feature partitions, T free]) so
projections consume the host-pre-transposed x^T directly, attention scores are
computed transposed (S^T[tk, tq]) so softmax(P)@V needs no transposes, and the
softmax denominator is broadcast for free by 64 ones-columns appended to V
(the PV matmul then emits 64 identical sum-exp rows; normalization is a copy +
one [64,1024] reciprocal + fused multiplies out of PSUM; reciprocal must read
SBUF - the custom-DVE op returns garbage on a partition-offset PSUM source).
Score matmuls are K=64 so the two heads' matmuls land on different PE row
groups (base partitions 0/64) and stream CONCURRENTLY (2x packing).

Schedule: the kernel is PE-streaming-bound (~224us of matmul columns at
2.4GHz) with the scalar engine (exp: (N+352)/1.2ns per instruction, ~163us
total) second. The emission is a dependency wavefront: x arrives on the two
HWDGE rings (sync+scalar — the only engines with hardware DGE); ~80 dummy
matmuls warm the PE HAM clock-gate (cold=1.2GHz) while the first slices land;
V/K/Q projections chase the slices; attention for query-tile qt starts as
soon as its K/Q columns exist. Attention inner loops pull cost-metered
"filler" PE work (remaining projections, Wo output blocks) from a queue
between score/PV pairs so the PE never idles while ACT chews exp: the ACT
deficit is ~1us per 2-key-block iteration. ACT runs nothing but exp — rope's
psum cast runs on DVE and rotate-half is a PE matmul against a constant
128x128 permutation (software-pipelined behind the cast via flush_rope, so
the PE never waits on DVE). qt region order 0,1,3,2 balances each region's
exp load against available filler (wave3 projections must drain during qt1
since qt3 needs them; wave2 defers into the qt3 region as filler); Wo(qt2)
plus its output DMA is the only tail. Output y rides the idle sync engine
only — a dma_start occupies its issuing engine ~5ns/descriptor, which would
starve exp if placed on scalar.

Note: the chip randomly enters a ~2.0GHz power state (vs 2.4) for whole runs;
measured times swing ~15% run-to-run on identical code.
"""

import numpy as np
import ml_dtypes
from collections import deque
from contextlib import ExitStack

import concourse.bass as bass
from concourse import bacc
import concourse.mybir as mybir
import concourse.tile as tile
from concourse.bass_utils import run_bass_kernel_spmd

BF16 = mybir.dt.bfloat16
F32 = mybir.dt.float32
AF = mybir.ActivationFunctionType

P = 128
B, T, D = 2, 2048, 2048
NUM_HEADS, NUM_KV_HEADS, HD = 32, 8, 64
FQ = 512          # q features per core (8 heads x 64)
DKV = 128         # kv features per core (2 kv heads x 64)
KO = D // P       # 16 contraction tiles over d_model
NT = T // 512     # 4 tiles of 512 along T
NXE = 8           # x arrives in 8 T-slices of 256
TE = T // NXE
SCALE = 1.0 / np.sqrt(HD)
ROPE_BASE = 10000.0
# local head order inside the 512 q-features: pairs (j, j+4) so that the two
# heads in partition tile j sit at bases 0/64 matching kv heads 0/1 in K^T
PERM_Q = [0, 4, 1, 5, 2, 6, 3, 7]

_nc_cache = {}


def build_nc():
    if "nc" in _nc_cache:
        return _nc_cache["nc"]
    nc = bacc.Bacc()
    # host-packed layouts: row = slice*128 + partition, all loads contiguous
    xS = nc.declare_dram_parameter("xS", [NXE * P, KO * TE], BF16, isOutput=False)
    wqS = nc.declare_dram_parameter("wqS", [4 * P, KO * P], BF16, isOutput=False)
    wkS = nc.declare_dram_parameter("wkS", [P, KO * DKV], BF16, isOutput=False)
    wvS = nc.declare_dram_parameter("wvS", [P, KO * DKV], BF16, isOutput=False)
    woS = nc.declare_dram_parameter("woS", [P, 4 * D], BF16, isOutput=False)
    cosd = nc.declare_dram_parameter("cosT", [P, T], BF16, isOutput=False)
    sind = nc.declare_dram_parameter("sinT", [P, T], BF16, isOutput=False)
    mskd = nc.declare_dram_parameter("tri", [P, P], BF16, isOutput=False)
    prmd = nc.declare_dram_parameter("prm", [P, P], BF16, isOutput=False)
    y = nc.declare_dram_parameter("y", [T, D], BF16, isOutput=True)

    with tile.TileContext(nc) as tc:
        with ExitStack() as ctx:
            const = ctx.enter_context(tc.tile_pool(name="const", bufs=1))
            work = ctx.enter_context(tc.tile_pool(name="work", bufs=6))
            otp = ctx.enter_context(tc.tile_pool(name="otp", bufs=2))
            pexp = ctx.enter_context(tc.tile_pool(name="pexp", bufs=8))
            rrp = ctx.enter_context(tc.tile_pool(name="rrp", bufs=2))
            ysp = ctx.enter_context(tc.tile_pool(name="ysp", bufs=2))
            big_ps = ctx.enter_context(tc.tile_pool(name="bigps", bufs=2, space="PSUM"))
            pv_ps = ctx.enter_context(tc.tile_pool(name="pvps", bufs=1, space="PSUM"))
            s_ps = ctx.enter_context(tc.tile_pool(name="sps", bufs=2, space="PSUM"))

            x_sb = const.tile([P, NXE, KO, TE], BF16, tag="x")
            wq_sb = const.tile([P, 4, KO, P], BF16, tag="wq")
            wv_sb = const.tile([P, KO, DKV], BF16, tag="wv")
            wk_sb = const.tile([P, KO, DKV], BF16, tag="wk")
            cos_sb = const.tile([P, T], BF16, tag="cos")
            sin_sb = const.tile([P, T], BF16, tag="sin")
            tri_sb = const.tile([P, P], BF16, tag="tri")
            prm_sb = const.tile([P, P], BF16, tag="prm")
            wo_sb = const.tile([P, 4, D], BF16, tag="wo")
            warm_sb = const.tile([P, P], BF16, tag="warm")

            # ---- V layout + PE warm-up (emitted first: gpsimd memsets, then
            # dummy matmuls keep the PE busy from ~6.4us so the HAM clock-gate
            # reaches 8/8 before real data lands, and stays there) ----
            v_sb = const.tile([P, 16, 256], BF16, tag="v")
            nc.gpsimd.memset(warm_sb[:], 0.0)
            nc.gpsimd.memset(v_sb[:, :, 64:128], 1.0)
            nc.gpsimd.memset(v_sb[:, :, 192:256], 1.0)
            wps = big_ps.tile([P, P], F32, tag="big")
            for _ in range(82):
                nc.tensor.matmul(wps[:], warm_sb[:], warm_sb[:],
                                 start=True, stop=True)

            # ---- input loads: two HWDGE rings (sync + gpsimd), x slices
            # split into partition halves so both rings carry every slice;
            # weights slotted just before first need ----
            def xs_half(e, h):
                return xS[e * P + 64 * h:e * P + 64 * (h + 1), :].rearrange(
                    "p (k t) -> p k t", k=KO)

            def wq_load(j):
                return wqS[j * P:(j + 1) * P, :].rearrange(
                    "p (k f) -> p k f", k=KO)

            def xs_full(e):
                return xS[e * P:(e + 1) * P, :].rearrange(
                    "p (k t) -> p k t", k=KO)

            # slices 0,1 split across both rings (halves the wave-0 latency);
            # later slices whole, alternating; weights slotted before first
            # need per the measured ~0.17 MB/us per-ring arrival rate
            def xs_q(e, q):
                return xS[e * P + 32 * q:e * P + 32 * (q + 1), :].rearrange(
                    "p (k t) -> p k t", k=KO)

            # slice 0 split into quarters across both rings so the first V
            # matmul can start ~14.4us instead of ~19; V/K-critical weights
            # ride the sync ring (it starts ~2us earlier)
            wv_src = wvS[:].rearrange("p (k f) -> p k f", k=KO)
            sync_q = [
                (x_sb[0:32, 0], xs_q(0, 0)),
                (x_sb[32:64, 0], xs_q(0, 1)),
                (wv_sb[0:64], wv_src[0:64]),
                (x_sb[0:64, 1], xs_half(1, 0)),
                (wk_sb[:], wkS[:].rearrange("p (k f) -> p k f", k=KO)),
                (wq_sb[:, 0], wq_load(0)),
                (wq_sb[:, 2], wq_load(2)),
                (x_sb[:, 2], xs_full(2)),
                (x_sb[:, 4], xs_full(4)),
                (x_sb[:, 6], xs_full(6)),
            ]
            scal_q = [
                (x_sb[64:96, 0], xs_q(0, 2)),
                (x_sb[96:128, 0], xs_q(0, 3)),
                (wv_sb[64:128], wv_src[64:128]),
                (x_sb[64:128, 1], xs_half(1, 1)),
                (prm_sb[:], prmd[:]),
                (cos_sb[:], cosd[:]),
                (sin_sb[:], sind[:]),
                (wq_sb[:, 1], wq_load(1)),
                (tri_sb[:], mskd[:]),
                (wq_sb[:, 3], wq_load(3)),
                (x_sb[:, 3], xs_full(3)),
                (x_sb[:, 5], xs_full(5)),
                (x_sb[:, 7], xs_full(7)),
                (wo_sb[:], woS[:].rearrange("p (k d) -> p k d", k=4)),
            ]
            # both HWDGE rings (only sync + scalar have them on trn2); the
            # scalar engine just fires the triggers up-front, before any exp
            for dst, src in sync_q:
                nc.sync.dma_start(dst, src)
            for dst, src in scal_q:
                nc.scalar.dma_start(dst, src)

            def x_mv(nt, ko):
                """[128, 2, 256] moving view of x tokens [nt*512,(nt+1)*512)"""
                return x_sb[:, 2 * nt:2 * nt + 2, ko, :]

            # rope: dst = raw*cos + rotate_half(raw)*sin. The rotate-half is
            # a PE matmul against a constant 128x128 permutation matrix
            # (rot_ps = PRM.T @ raw, 213ns) — no DMA ring traffic, no scalar
            # engine. It reads the bf16 cast, so it is software-pipelined:
            # the perm matmul + combine of rope i are emitted at the start
            # of the NEXT unit (flush_rope), hiding the DVE-cast latency.
            rope_pending = []

            def flush_rope():
                while rope_pending:
                    rope_pending.pop(0)()

            def rope(dst, nt):
                ts = slice(nt * 512, (nt + 1) * 512)

                def fin(ps):
                    raw = work.tile([P, 512], BF16, tag="ropraw")
                    nc.vector.tensor_copy(raw[:], ps[:])

                    def finish():
                        rps = big_ps.tile([P, 512], F32, tag="big")
                        nc.tensor.matmul(rps[:], prm_sb[:], raw[:],
                                         start=True, stop=True)
                        t1 = work.tile([P, 512], BF16, tag="ropt1")
                        nc.vector.tensor_mul(t1[:], raw[:], cos_sb[:, ts])
                        rtb = work.tile([P, 512], BF16, tag="roprtb")
                        nc.vector.tensor_mul(rtb[:], rps[:], sin_sb[:, ts])
                        nc.vector.tensor_add(dst[:, ts], t1[:], rtb[:])
                    rope_pending.append(finish)
                return fin

            # ---- K projection + rope (feature-major K^T [128, T]) ----
            kt = const.tile([P, T], BF16, tag="kt")

            def k_proj(nt):
                flush_rope()
                ps = big_ps.tile([P, 512], F32, tag="big")
                for ko in range(KO):
                    nc.tensor.matmul(ps[:], wk_sb[:, ko, :], x_mv(nt, ko),
                                     start=(ko == 0), stop=(ko == KO - 1))
                rope(kt, nt)(ps)

            # ---- V projection (token-major, 64 ones columns per head) ----
            def v_proj(tt):
                flush_rope()
                ps = big_ps.tile([P, DKV], F32, tag="big")
                for ko in range(KO):
                    nc.tensor.matmul(
                        ps[:], x_sb[:, tt // 2, ko,
                                    (tt % 2) * P:(tt % 2) * P + P],
                        wv_sb[:, ko, :],
                        start=(ko == 0), stop=(ko == KO - 1))
                nc.vector.tensor_copy(v_sb[:, tt, 0:64], ps[:, 0:64])
                nc.vector.tensor_copy(v_sb[:, tt, 128:192], ps[:, 64:128])

            # ---- Q projection + rope for one head pair, one token tile ----
            qts = {}
            for j in range(4):
                qts[j] = const.tile([P, T], BF16, tag=f"qt{j}", name=f"qt{j}")

            def q_proj_nt(j, nt):
                flush_rope()
                ps = big_ps.tile([P, 512], F32, tag="big")
                for ko in range(KO):
                    nc.tensor.matmul(ps[:], wq_sb[:, j, ko, :], x_mv(nt, ko),
                                     start=(ko == 0), stop=(ko == KO - 1))
                rope(qts[j], nt)(ps)

            # ---- filler queue: PE work pulled between attention pairs.
            # Entries carry an estimated PE cost (us); pull(budget) drains
            # ~budget worth of work, carrying surplus credit so chunky units
            # (3.4us q-projections) average out over iterations ----
            pool = deque()
            pull_credit = [0.0]

            def pull(budget):
                pull_credit[0] += budget
                while pool and pull_credit[0] > 0:
                    cost, fn = pool.popleft()
                    fn()
                    pull_credit[0] -= cost

            def drain():
                while pool:
                    pool.popleft()[1]()
                pull_credit[0] = 0.0

            # ---- attention for one (qt, j) head-pair into ot tile ----
            def attn_block(qt, j, ot, split_epi=False, pull_budget=1.0):
                flush_rope()
                pv = pv_ps.tile([P, 1024], F32, tag="pv")
                nkb = 4 * qt + 4

                def flush_pv(prev):
                    # PV matmuls for the previous kb (software pipeline: issued
                    # after the next kb's scores so PE never waits on ACT's exp
                    # of the current block). Diagonal blocks only touch output
                    # columns >= their first causally-valid query. (Splitting
                    # each PV into two concurrent K=64 row-tiles crashes the
                    # device - two in-flight matmuls may not share a psum bank.)
                    pkb, c0, pp = prev
                    ppv = pp[:].rearrange("p (two t) -> p two t", two=2)
                    nc.tensor.matmul(pv[:, c0:512], v_sb[:, pkb, 0:128],
                                     ppv[:, 0, c0:512],
                                     start=(pkb == 0), stop=(pkb == nkb - 1))
                    nc.tensor.matmul(pv[:, 512 + c0:1024], v_sb[:, pkb, 128:256],
                                     ppv[:, 1, c0:512],
                                     start=(pkb == 0), stop=(pkb == nkb - 1))

                pending = []
                for kb0 in range(0, nkb, 2):
                    # issue TWO key-blocks' score pairs back-to-back: the
                    # later kt LDWEIGHTS pull ahead behind score matmuls in
                    # the other row group (a full-row PV matmul in between
                    # would block the pull-ahead and expose ~107ns each)
                    sps = []
                    for kb in (kb0, kb0 + 1):
                        tk = slice(kb * P, (kb + 1) * P)
                        jr = kb - 4 * qt       # >= 0 on diagonal blocks
                        c0 = max(0, jr) * P    # first causally-valid column
                        tqs = slice(qt * 512 + c0, (qt + 1) * 512)
                        # one 2-bank psum tile holds both heads' scores; the
                        # two matmuls write disjoint banks, then a SINGLE exp
                        # (3-dim AP) and a single broadcast mask cover both
                        # halves, halving the pacing-engine instruction count
                        sp = s_ps.tile([P, 1024], F32, tag="s")
                        spv = sp[:].rearrange("p (two t) -> p two t", two=2)
                        nc.tensor.matmul(sp[:, c0:512], kt[0:64, tk],
                                         qts[j][0:64, tqs],
                                         start=True, stop=True)
                        nc.tensor.matmul(sp[:, 512 + c0:1024], kt[64:128, tk],
                                         qts[j][64:128, tqs],
                                         start=True, stop=True)
                        sps.append((kb, c0, jr, spv))
                    while pending:
                        flush_pv(pending.pop(0))
                    for kb, c0, jr, spv in sps:
                        pp = pexp.tile([P, 1024], BF16, tag="p")
                        ppv = pp[:].rearrange("p (two t) -> p two t", two=2)
                        nc.scalar.activation(ppv[:, :, c0:512],
                                             spv[:, :, c0:512],
                                             AF.Exp, scale=SCALE)
                        if jr >= 0:
                            # triangle mask on the partially-valid block
                            nc.vector.tensor_mul(
                                ppv[:, :, c0:c0 + P], ppv[:, :, c0:c0 + P],
                                tri_sb[:, None, :].to_broadcast((P, 2, P)))
                        pending.append((kb, c0, pp))
                    pull(pull_budget)
                for pr in pending:
                    flush_pv(pr)
                # normalization: rows 64..127 of pv hold 64 copies of the
                # sum-exp row (ones trick): stage to SBUF, one wide
                # reciprocal, then fused multiplies finalize ot from PSUM
                den = rrp.tile([64, 1024], F32, tag="den")
                rec = rrp.tile([64, 1024], F32, tag="rec")
                if split_epi:
                    # per-head chain on the tail block: head0's normalize
                    # overlaps head1's final PV matmul
                    for hh in range(2):
                        cs = slice(hh * 512, hh * 512 + 512)
                        nc.vector.tensor_copy(den[:, cs], pv[64:128, cs])
                        nc.vector.reciprocal_approx_fast(rec[:, cs],
                                                         den[:, cs])
                        nc.vector.tensor_mul(ot[hh * 64:hh * 64 + 64, j, :],
                                             pv[0:64, cs], rec[:, cs])
                else:
                    nc.vector.tensor_copy(den[:], pv[64:128, :])
                    nc.vector.reciprocal_approx_fast(rec[:], den[:])
                    nc.vector.tensor_mul(ot[0:64, j, :], pv[0:64, 0:512],
                                         rec[:, 0:512])
                    nc.vector.tensor_mul(ot[64:128, j, :], pv[0:64, 512:1024],
                                         rec[:, 512:1024])

            # ---- Wo output projection, emitted as per-oc filler units ----
            ysbs = {}

            def wo_unit(qt, tt, oc, ot):
                flush_rope()
                r0 = qt * 512 + tt * P
                if oc == 0:
                    ysbs[(qt, tt)] = ysp.tile([P, D], BF16, tag="ysb",
                                              name=f"ysb{qt}_{tt}")
                ysb = ysbs[(qt, tt)]
                yps = big_ps.tile([P, 512], F32, tag="big")
                for kf in range(4):
                    nc.tensor.matmul(yps[:], ot[:, kf, tt * P:(tt + 1) * P],
                                     wo_sb[:, kf, oc * 512:(oc + 1) * 512],
                                     start=(kf == 0), stop=(kf == 3))
                nc.vector.tensor_copy(ysb[:, oc * 512:(oc + 1) * 512], yps[:])
                if qt == 2:
                    # tail blocks: ship each oc chunk immediately so the
                    # final drain is one 128KB chunk, not a 0.5MB row
                    nc.sync.dma_start(y[r0:r0 + P, oc * 512:(oc + 1) * 512],
                                      ysb[:, oc * 512:(oc + 1) * 512])
                elif oc == 3:
                    # one whole-row DMA (4KB descriptors) on the otherwise
                    # idle sync engine; scalar must stay free for exp
                    nc.sync.dma_start(y[r0:r0 + P, :], ysb[:])

            def queue_wo(qt):
                ot = ot_tiles[qt]
                for tt in range(4):
                    for oc in range(4):
                        pool.append((0.85,
                                     lambda qt=qt, tt=tt, oc=oc, ot=ot:
                                     wo_unit(qt, tt, oc, ot)))

            # ---- emission: dependency wavefront ----
            # wave 0 (x slices 0,1 + wk/wv/wq/cos/sin): V, K, all-j Q for nt0
            v_proj(0); v_proj(1); v_proj(2); v_proj(3)
            k_proj(0)
            # j order matches wq arrival: wq0/wq2 ride the sync ring, wq1/wq3
            # land later on the scalar ring behind the rope tables
            for j in (0, 2, 1, 3):
                q_proj_nt(j, 0)

            ot_tiles = {qt: otp.tile([P, 4, 512], BF16, tag="ot",
                                     name=f"ot{qt}") for qt in range(4)}

            # region 1: attn qt0; filler = wave 1 (x slices 2,3)
            for f in [(0.9, lambda: v_proj(4)), (0.9, lambda: v_proj(5)),
                      (0.9, lambda: v_proj(6)), (0.9, lambda: v_proj(7)),
                      (3.4, lambda: k_proj(1)),
                      (3.4, lambda: q_proj_nt(0, 1)),
                      (3.4, lambda: q_proj_nt(1, 1)),
                      (3.4, lambda: q_proj_nt(2, 1)),
                      (3.4, lambda: q_proj_nt(3, 1))]:
                pool.append(f)
            for j in range(4):
                attn_block(0, j, ot_tiles[0])
                pull(1.0)
            drain()             # qt1 needs all of wave 1

            # region 2: attn qt1; filler = wave 3 (x slices 6,7), which must
            # fully precede qt3's attention, so it drains here
            for f in [(0.9, lambda: v_proj(12)), (0.9, lambda: v_proj(13)),
                      (0.9, lambda: v_proj(14)), (0.9, lambda: v_proj(15)),
                      (3.4, lambda: k_proj(3)),
                      (3.4, lambda: q_proj_nt(0, 3)),
                      (3.4, lambda: q_proj_nt(1, 3)),
                      (3.4, lambda: q_proj_nt(2, 3)),
                      (3.4, lambda: q_proj_nt(3, 3))]:
                pool.append(f)
            for j in range(4):
                attn_block(1, j, ot_tiles[1])
                pull(1.0)
            drain()

            # region 3: attn qt3 (heaviest exp load). Filler = wave 2 (only
            # needed by qt2, i.e. region 4) + Wo(qt0): enough PE work that
            # the scores never stall on the exp double-buffer.
            for f in [(0.9, lambda: v_proj(8)), (0.9, lambda: v_proj(9)),
                      (0.9, lambda: v_proj(10)), (0.9, lambda: v_proj(11)),
                      (3.4, lambda: k_proj(2)),
                      (3.4, lambda: q_proj_nt(0, 2)),
                      (3.4, lambda: q_proj_nt(1, 2)),
                      (3.4, lambda: q_proj_nt(2, 2)),
                      (3.4, lambda: q_proj_nt(3, 2))]:
                pool.append(f)
            queue_wo(0)
            for j in range(4):
                attn_block(3, j, ot_tiles[3])
                pull(1.0)
            drain()

            # region 4: attn qt2; filler = Wo(qt1) + Wo(qt3)
            queue_wo(1)
            queue_wo(3)
            for j in range(4):
                attn_block(2, j, ot_tiles[2],
                           split_epi=(j == 3), pull_budget=0.8)
                pull(0.8)
            # leftovers (a few Wo(qt1/qt3) units) run HERE, covering the PE
            # while attn(2,3)'s normalize chain drains on the vector engine —
            # otherwise the first Wo(qt2) LDW stalls ~4.4us on it
            drain()

            # tail: Wo(qt2)
            queue_wo(2)
            drain()

    nc.finalize()
    _nc_cache["nc"] = nc
    return nc


def make_in_maps(x, Wq, Wk, Wv, Wo):
    bf = ml_dtypes.bfloat16
    x = np.asarray(x, np.float32)
    Wq = np.asarray(Wq, np.float32)
    Wk = np.asarray(Wk, np.float32)
    Wv = np.asarray(Wv, np.float32)
    Wo = np.asarray(Wo, np.float32)

    # rope tables, [128, T]: row p covers head-dim d = p % 64
    half = HD // 2
    inv_freq = 1.0 / (ROPE_BASE ** (np.arange(half, dtype=np.float64) / half))
    pos = np.arange(T, dtype=np.float64)
    d_idx = np.arange(P) % HD
    freqs = pos[None, :] * inv_freq[d_idx % half][:, None]      # [128, T]
    cos_t = np.cos(freqs).astype(bf)
    sign = np.where(d_idx < half, -1.0, 1.0)[:, None]
    sin_t = (np.sin(freqs) * sign).astype(bf)

    # causal 0/1 triangle for the partially-valid diagonal sub-block
    pp = np.arange(P)[:, None]
    ff = np.arange(P)[None, :]
    tri = (ff >= pp).astype(bf)

    # rotate-half permutation matrix: prm[k, m] = 1 iff k == rot(m), so the
    # PE matmul prm.T @ raw yields raw[rot(m)] on partition m (sign lives in
    # the sin table)
    m_idx = np.arange(P)
    rot_m = np.where(m_idx % HD < half, m_idx + half, m_idx - half)
    prm = np.zeros((P, P), np.float32)
    prm[rot_m, m_idx] = 1.0
    prm = prm.astype(bf)

    def pack(a, n_chunks):
        # [n_chunks*128, F] -> [128, n_chunks*F] partition-major
        F = a.shape[1]
        return np.ascontiguousarray(
            a.reshape(n_chunks, P, F).transpose(1, 0, 2).reshape(P, n_chunks * F))

    in_maps = []
    for c in range(8):
        b, g = c // 4, c % 4
        heads = [8 * g + h for h in PERM_Q]
        qrows = np.concatenate([np.arange(h * HD, (h + 1) * HD) for h in heads])
        kvrows = np.arange(2 * g * HD, (2 * g + 2) * HD)
        xT = np.ascontiguousarray(x[b].T).astype(bf)             # [D, T]
        wqT = np.ascontiguousarray(Wq[qrows, :].T).astype(bf)    # [D, FQ]
        wkT = np.ascontiguousarray(Wk[kvrows, :].T).astype(bf)
        wvT = np.ascontiguousarray(Wv[kvrows, :].T).astype(bf)
        woT = np.ascontiguousarray(Wo[:, qrows].T).astype(bf)    # [FQ, D]
        # xS rows = e*128 + p, cols = ko*TE + t  (slice e, token e*TE+t)
        xs = np.ascontiguousarray(
            xT.reshape(KO, P, NXE, TE).transpose(2, 1, 0, 3)
            .reshape(NXE * P, KO * TE))
        # wqS rows = j*128 + p, cols = ko*128 + f
        wqs = np.ascontiguousarray(
            wqT.reshape(KO, P, 4, P).transpose(2, 1, 0, 3).reshape(4 * P, KO * P))
        in_maps.append({
            "xS": xs,
            "wqS": wqs,
            "wkS": pack(wkT, KO),
            "wvS": pack(wvT, KO),
            "woS": pack(woT, 4),
            "cosT": cos_t,
            "sinT": sin_t,
            "tri": tri,
            "prm": prm,
        })
    return in_maps


def combine_outputs(results):
    out = np.zeros((B, T, D), np.float32)
    for c in range(8):
        out[c // 4] += np.asarray(results[c]["y"], np.float32)
    return out


def _ensure_ntff_hook():
    """Register the axon NTFF profile hook (antenv.axon_hooks is missing
    from this image; recreate it and wire the ctypes hook from trn_boot)."""
    import sys, types
    if "antenv.axon_hooks" in sys.modules:
        return
    m = types.ModuleType("antenv.axon_hooks")
    hook = [None]
    m.set_axon_ntff_profile_hook = lambda h: hook.__setitem__(0, h)
    m.get_axon_ntff_profile_hook = lambda: hook[0]
    sys.modules["antenv.axon_hooks"] = m
    import antenv
    antenv.axon_hooks = m
    sys.path.insert(0, "/root/.axon_site")
    from trn_agent_boot.trn_boot import _ntff_profile_via_ctypes
    m.set_axon_ntff_profile_hook(
        _ntff_profile_via_ctypes("/opt/axon/libaxon_pjrt.so"))


def kernel(x, Wq, Wk, Wv, Wo, _trace=False):
    if _trace:
        _ensure_ntff_hook()
    nc = build_nc()
    in_maps = make_in_maps(x, Wq, Wk, Wv, Wo)
    res = run_bass_kernel_spmd(nc, in_maps, core_ids=list(range(8)), trace=_trace)
    out = combine_outputs(res.results)
    if _trace:
        return out, res
    return out
